# revision 1
# baseline (speedup 1.0000x reference)
"""CrossTeacherAttention Trainium2 kernel.

Per batch element b (x as [C=256, N=1024], N=H*W):
  Q = Wq @ Xs + bq  [C,N];  K_t = Wk @ Xt_t + bk  [C,N]
  Vt^T = Xt_t^T @ Wv^T  [N,C]  (bv deferred to the end)
  S_t^T[m,n] = sum_c K_t[c,m] Q[c,n];  E_t = exp(S_t^T/16)
  Z_t[n] = sum_m E_t[m,n];  O_t^T[c,n] = sum_m Vt^T[m,c] E_t[m,n] / Z_t[n]
  out = Xs + bv + (1/3) sum_t O_t^T
attn.mean(-1) of a softmax is exactly 1/N, so the teacher weights are
uniformly 1/3; folded with 1/Z_t into one reciprocal (ones-vector of 3.0
in the Z row-sum matmul), applied to E_t before the O matmuls so all
teachers accumulate into one PSUM region. Matmuls run in float32r (full
PE rate; plain fp32 takes 2 half-speed passes) with producers rounding
explicitly. Softmax max-subtraction skipped: |S/16| <~ 7 for this regime.

Sharding: data-parallel over batch, B=8 -> one batch element per core.
"""

import sys

sys.path.insert(0, "/opt/trn_rl_repo")

import numpy as np

import concourse.bass as bass
import concourse.tile as tile
from concourse import mybir
from concourse.bass_utils import run_bass_kernel_spmd

B, C, H, W = 8, 256, 32, 32
N = H * W  # 1024
T = 3
P = 128
CC = C // P  # 2 c-chunks
MC = N // P  # 8 m-chunks
NH = N // 512  # 2 n-halves
F32 = mybir.dt.float32
F32R = mybir.dt.float32r
SCALE = C ** -0.5  # 1/16


def build_nc():
    nc = bass.Bass()
    xs_d = nc.dram_tensor("xs", [C, N], F32, kind="ExternalInput")
    xt_d = nc.dram_tensor("xt", [T, C, N], F32, kind="ExternalInput")
    wqT_d = nc.dram_tensor("wqT", [C, C], F32, kind="ExternalInput")
    wkT_d = nc.dram_tensor("wkT", [C, C], F32, kind="ExternalInput")
    wvT_d = nc.dram_tensor("wvT", [C, C], F32, kind="ExternalInput")
    bq_d = nc.dram_tensor("bq", [C, 1], F32, kind="ExternalInput")
    bk_d = nc.dram_tensor("bk", [C, 1], F32, kind="ExternalInput")
    bv_d = nc.dram_tensor("bv", [C, 1], F32, kind="ExternalInput")
    out_d = nc.dram_tensor("out", [C, N], F32, kind="ExternalOutput")

    with tile.TileContext(nc) as tc:
        with (
            tc.tile_pool(name="consts", bufs=1) as consts,
            tc.tile_pool(name="ldpool", bufs=2) as ldpool,
            tc.tile_pool(name="kpool", bufs=6) as kpool,
            tc.tile_pool(name="vpool", bufs=24) as vpool,
            tc.tile_pool(name="epool", bufs=10) as epool,
            tc.tile_pool(name="rpool", bufs=1) as rpool,
            tc.tile_pool(name="bpool", bufs=2) as bpool,
            tc.tile_pool(name="tpool", bufs=2) as tpool,
            tc.tile_pool(name="opool", bufs=2) as opool,
            tc.tile_pool(name="ps", bufs=4, space="PSUM") as ps,
            tc.tile_pool(name="po", bufs=2, space="PSUM") as po,
            tc.tile_pool(name="zps", bufs=2, space="PSUM") as zps,
        ):
            # ---- loads + one-time rounding copies to float32r ----
            def load_r(dram_ap, shape, tag, keep_f32=False, conv_act=False):
                ld = ldpool.tile(shape, F32, tag=f"ld{shape[1]}", name=f"ld_{tag}")
                nc.sync.dma_start(out=ld, in_=dram_ap)
                rt = consts.tile(shape, F32R, tag=tag, name=f"r_{tag}")
                if conv_act:
                    nc.scalar.copy(rt, ld)
                else:
                    nc.vector.tensor_copy(rt, ld)
                if keep_f32:
                    ft = consts.tile(shape, F32, tag=f"f{tag}", name=f"f_{tag}")
                    nc.vector.tensor_copy(ft, ld)
                    return rt, ft
                return rt

            xs_r, xs_sb = [], []
            wqT_r, wkT_r, wvT_r = [], [], []
            bq_sb, bk_sb, bv_sb = [], [], []
            for ci in range(CC):
                sl = slice(ci * P, (ci + 1) * P)
                rt, ft = load_r(xs_d[sl, :], [P, N], f"xs{ci}", keep_f32=True,
                                conv_act=False)
                xs_r.append(rt)
                xs_sb.append(ft)
                wqT_r.append(load_r(wqT_d[sl, :], [P, C], f"wq{ci}"))
                wkT_r.append(load_r(wkT_d[sl, :], [P, C], f"wk{ci}"))
                wvT_r.append(load_r(wvT_d[sl, :], [P, C], f"wv{ci}"))
                for lst, dram, tg in (
                    (bq_sb, bq_d, "bq"), (bk_sb, bk_d, "bk"), (bv_sb, bv_d, "bv"),
                ):
                    b_ = consts.tile([P, 1], F32, tag=f"{tg}{ci}", name=f"{tg}{ci}")
                    nc.sync.dma_start(out=b_, in_=dram[sl, :])
                    lst.append(b_)
            xt_r = [[load_r(xt_d[t, ci * P:(ci + 1) * P, :], [P, N],
                            f"xt{t}{ci}", conv_act=False) for ci in range(CC)]
                    for t in range(T)]
            ones3 = consts.tile([P, 1], F32, tag="ones3", name="ones3")
            nc.vector.memset(ones3, 3.0)
            ones3r = consts.tile([P, 1], F32R, tag="ones3r", name="ones3r")
            nc.vector.tensor_copy(ones3r, ones3)
            ones_row = consts.tile([1, P], F32, tag="ones_row", name="ones_row")
            nc.vector.memset(ones_row, 1.0)
            ones_rowr = consts.tile([1, P], F32R, tag="ones_rowr",
                                    name="ones_rowr")
            nc.vector.tensor_copy(ones_rowr, ones_row)

            # ---- running output accumulator: acc = xs + bv ----
            acc = []
            for co in range(CC):
                a_ = consts.tile([P, N], F32, tag=f"acc{co}", name=f"acc{co}")
                nc.vector.tensor_scalar_add(a_, xs_sb[co], bv_sb[co])
                acc.append(a_)

            # ---- Q projection: Q[c,n] (float32r output for the S matmuls) ----
            q_sb = []
            for co in range(CC):
                qt = consts.tile([P, N], F32R, tag=f"q{co}", name=f"q{co}")
                for nh in range(NH):
                    qp = ps.tile([P, 512], F32, tag="ps", name="qp")
                    for ci in range(CC):
                        nc.tensor.matmul(
                            qp,
                            wqT_r[ci][:, co * P:(co + 1) * P],
                            xs_r[ci][:, nh * 512:(nh + 1) * 512],
                            start=(ci == 0),
                            stop=(ci == CC - 1),
                        )
                    nc.vector.tensor_scalar_add(
                        qt[:, nh * 512:(nh + 1) * 512], qp, bq_sb[co]
                    )
                q_sb.append(qt)

            # ---- all teachers' K and V^T projections up front ----
            k_all, v_all = [], []
            for t in range(T):
                k_sb = []
                for co in range(CC):
                    kt = kpool.tile([P, N], F32R, tag="k", name=f"k{t}{co}")
                    for nh in range(NH):
                        kp = ps.tile([P, 512], F32, tag="ps", name="kp")
                        for ci in range(CC):
                            nc.tensor.matmul(
                                kp,
                                wkT_r[ci][:, co * P:(co + 1) * P],
                                xt_r[t][ci][:, nh * 512:(nh + 1) * 512],
                                start=(ci == 0),
                                stop=(ci == CC - 1),
                            )
                        nc.vector.tensor_scalar_add(
                            kt[:, nh * 512:(nh + 1) * 512], kp, bk_sb[co]
                        )
                    k_sb.append(kt)
                k_all.append(k_sb)
                vT = []
                for mi in range(MC):
                    vp = ps.tile([P, C], F32, tag="ps", name="vp")
                    for ci in range(CC):
                        nc.tensor.matmul(
                            vp,
                            xt_r[t][ci][:, mi * P:(mi + 1) * P],
                            wvT_r[ci],
                            start=(ci == 0),
                            stop=(ci == CC - 1),
                        )
                    vt_ = vpool.tile([P, C], F32R, tag="v", name=f"v{t}{mi}")
                    nc.any.tensor_copy(vt_, vp)
                    vT.append(vt_)
                v_all.append(vT)

            for t in range(T):
                k_sb = k_all[t]
                vT = v_all[t]
                # per-teacher PSUM accumulators: Z rows; O done per c-chunk
                zpt = [zps.tile([1, 512], F32, tag="zp", name=f"zp{t}{nh}")
                       for nh in range(NH)]
                # S^T -> exp(float32r) -> e; Z matmuls consume e directly
                e = []
                for mi in range(MC):
                    et = epool.tile([P, N], F32R, tag="e", name=f"e{t}{mi}")
                    for nh in range(NH):
                        sp = ps.tile([P, 512], F32, tag="ps", name="sp")
                        for ci in range(CC):
                            nc.tensor.matmul(
                                sp,
                                k_sb[ci][:, mi * P:(mi + 1) * P],
                                q_sb[ci][:, nh * 512:(nh + 1) * 512],
                                start=(ci == 0),
                                stop=(ci == CC - 1),
                            )
                        nc.scalar.activation(
                            et[:, nh * 512:(nh + 1) * 512],
                            sp,
                            func=mybir.ActivationFunctionType.Exp,
                            scale=SCALE,
                        )
                    e.append(et)
                    for nh in range(NH):
                        nc.tensor.matmul(
                            zpt[nh], ones3r,
                            et[:, nh * 512:(nh + 1) * 512],
                            start=(mi == 0), stop=(mi == MC - 1),
                        )
                # recipZ = 1/(3 Z); broadcast along partitions via DMA
                recip = rpool.tile([1, N], F32, tag="r", name=f"recip{t}")
                for nh in range(NH):
                    nc.vector.reciprocal(
                        recip[:, nh * 512:(nh + 1) * 512], zpt[nh]
                    )
                recipr = rpool.tile([1, N], F32R, tag="rr", name=f"recipr{t}")
                nc.vector.tensor_copy(recipr, recip)
                bcast = bpool.tile([P, N], F32, tag="b", name=f"bcast{t}")
                for nh in range(NH):
                    bp = ps.tile([P, 512], F32, tag="ps", name="bp")
                    nc.tensor.matmul(
                        bp, ones_rowr, recipr[:, nh * 512:(nh + 1) * 512],
                        start=True, stop=True,
                    )
                    nc.vector.tensor_copy(
                        bcast[:, nh * 512:(nh + 1) * 512], bp)
                # O accumulation per c-chunk, then late normalization:
                # acc += O_t[co] * bcast
                for co in range(CC):
                    otp = [po.tile([P, 512], F32, tag="po", name=f"ot{t}{co}{nh}")
                           for nh in range(NH)]
                    for mi in range(MC):
                        for nh in range(NH):
                            nc.tensor.matmul(
                                otp[nh],
                                vT[mi][:, co * P:(co + 1) * P],
                                e[mi][:, nh * 512:(nh + 1) * 512],
                                start=(mi == 0),
                                stop=(mi == MC - 1),
                            )
                    tmp = tpool.tile([P, N], F32, tag="tmp", name=f"tmp{t}{co}")
                    for nh in range(NH):
                        nc.vector.tensor_mul(
                            tmp[:, nh * 512:(nh + 1) * 512],
                            otp[nh],
                            bcast[:, nh * 512:(nh + 1) * 512],
                        )
                    nc.vector.tensor_add(acc[co], acc[co], tmp)

            # ---- store straight from the accumulators ----
            for co in range(CC):
                nc.sync.dma_start(out=out_d[co * P:(co + 1) * P, :], in_=acc[co])

    _split_multi_waits(nc)
    if not nc.is_finalized():
        nc.finalize()
    return nc


def _split_multi_waits(nc):
    """walrus can encode at most one sync-wait per instruction. Hoist every
    wait of a multi-wait instruction onto single-wait nops on the same
    engine, placed immediately before it in program order."""
    fixes = []
    for fn in nc.m.functions:
        for blk in fn.blocks:
            for inst in blk.instructions:
                si = getattr(inst, "sync_info", None)
                if (si is not None and si.on_wait and len(si.on_wait) > 1
                        and getattr(inst, "engine", None) is not None):
                    fixes.append((blk, inst))
    for blk, inst in fixes:
        si = inst.sync_info
        waits = list(si.on_wait)
        nops = []
        for w in waits:
            nop = nc.engines[inst.engine].nop(nofuse=True).ins
            nop.sync_info = mybir.SyncInfo(on_wait=[w], on_update=[])
            nops.append(nop)
        inst.sync_info = mybir.SyncInfo(on_wait=[], on_update=list(si.on_update))
        nop_names = {n.name for n in nops}
        for fn2 in nc.m.functions:
            for blk2 in fn2.blocks:
                blk2.instructions = [
                    i for i in blk2.instructions if i.name not in nop_names
                ]
        pos = next(i for i, x in enumerate(blk.instructions)
                   if x.name == inst.name)
        blk.instructions = (blk.instructions[:pos] + nops
                            + blk.instructions[pos:])


_NC = None


def _get_nc():
    global _NC
    if _NC is None:
        _NC = build_nc()
    return _NC


def make_in_maps(student_feat, t_feat0, t_feat1, t_feat2,
                 Wq, bq, Wk, bk, Wv, bv):
    xs = np.ascontiguousarray(student_feat.reshape(B, C, N), dtype=np.float32)
    xt = np.ascontiguousarray(
        np.stack([t_feat0, t_feat1, t_feat2], axis=1).reshape(B, T, C, N),
        dtype=np.float32)
    wqT = np.ascontiguousarray(Wq.T, dtype=np.float32)
    wkT = np.ascontiguousarray(Wk.T, dtype=np.float32)
    wvT = np.ascontiguousarray(Wv.T, dtype=np.float32)
    bqc = np.ascontiguousarray(bq.reshape(C, 1), dtype=np.float32)
    bkc = np.ascontiguousarray(bk.reshape(C, 1), dtype=np.float32)
    bvc = np.ascontiguousarray(bv.reshape(C, 1), dtype=np.float32)
    return [
        {"xs": xs[b], "xt": xt[b], "wqT": wqT, "wkT": wkT, "wvT": wvT,
         "bq": bqc, "bk": bkc, "bv": bvc}
        for b in range(B)
    ]


def run(in_maps, trace=False):
    nc = _get_nc()
    return run_bass_kernel_spmd(nc, in_maps, core_ids=list(range(B)),
                                trace=trace)


def kernel(student_feat, t_feat0, t_feat1, t_feat2,
           Wq, bq, Wk, bk, Wv, bv):
    in_maps = make_in_maps(student_feat, t_feat0, t_feat1, t_feat2,
                           Wq, bq, Wk, bk, Wv, bv)
    res = run(in_maps, trace=False)
    out = np.stack([res.results[b]["out"].reshape(C, H, W) for b in range(B)])
    return out.astype(np.float32)



# revision 9
# speedup vs baseline: 2.2276x; 2.2276x over previous
"""CrossTeacherAttention Trainium2 kernel (v2).

Math (per batch element b; xs/xt as [C=256, N=1024], N=H*W):
  S_t[n,m] = scale * (q_n . k_m) with q = Wq xs + bq, k_t = Wk xt_t + bk.
  bk-terms are constant per softmax column -> dropped (softmax invariant).
  S_t = xs^T (Wq^T Wk) xt_t + (Wk^T bq)^T xt_t.  Host folds the weights:
    G16 = 16 Wq^T Wk,  u16 = 16 Wk^T bq  (x16 keeps fp8 entries in normal
    range), so with P16 = G16^T xs the exp argument is
    (xt^T P16)[m,n]/256 + kb_t[m]  with kb_t = (xt_t^T u16)/256 - 2
  (-2 is a uniform shift so exp output fits fp8; cancels in softmax).
  attn.mean(-1) of a softmax is exactly 1/N -> teacher weights are 1/3.
  out = xs + bv + sum_t (E'_t V_t) / (3 Z'_t),  Z'_t[n] = sum_m E'_t[m,n].

Layout trick: O is computed with E' as the *stationary* matmul operand
(lhsT = E'[m,2,n], rhs = V^T[m,2,c]) so the output lands as [n_part, c_free],
where the softmax normalizer is a per-partition scalar.  V^T gets a 257th
column holding 48.0, so column 256 of the O accumulator is 48*Z'_t and
(1/48)*16(V-scale) = 1/3 folds the teacher weight into the reciprocal.
Z row-sums and the bias fold thus cost no extra PE streaming.

All matmuls run in fp8e4 DoubleRow (two 128-row k-tiles per pass).  exp runs
on ACT reading [128,1024] PSUM spans, writing fp8 E' directly.  Inputs are
pre-quantized/interleaved on the host ([128, 2, *] k-pair layout), which also
halves HBM traffic.  Residual path (xs^T + bv) ships as bf16; output is bf16
[n, c], transposed back on the host.

Sharding: data-parallel over batch, B=8 -> one batch element per core.
"""

import sys

sys.path.insert(0, "/opt/trn_rl_repo")

import ml_dtypes
import numpy as np

import concourse.bass as bass
import concourse.tile as tile
from concourse import mybir
from concourse.bass_utils import run_bass_kernel_spmd

B, C, H, W = 8, 256, 32, 32
N = H * W  # 1024
T = 3
P = 128
NCH = N // P  # 8 n-chunks
MC = N // P  # 8 m-chunks
PAIRS = MC // 2  # 4 m-chunk pairs (DoubleRow)
F32 = mybir.dt.float32
BF16 = mybir.dt.bfloat16
F8 = mybir.dt.float8e4
F8E5 = mybir.dt.float8e5
DR = mybir.MatmulPerfMode.DoubleRow
SCALE = 1.0 / 256.0  # 1/sqrt(C) / 16 (the x16 weight prescale)
EXP_SHIFT = -2.0
ZCOL = 48.0  # 16 (V prescale) * 3 (teacher weight)
AOT = mybir.AluOpType


def build_nc():
    nc = bass.Bass()
    xs8_d = nc.dram_tensor("xs8", [P, 2, N], F8, kind="ExternalInput")
    g8_d = nc.dram_tensor("g8", [P, 2, C], F8, kind="ExternalInput")
    xt8_d = nc.dram_tensor("xt8", [T, P, 2, N], F8, kind="ExternalInput")
    wv8_d = nc.dram_tensor("wv8", [P, 2, C], F8, kind="ExternalInput")
    u8_d = nc.dram_tensor("u8", [P, 2, 1], F8, kind="ExternalInput")
    xsbv_d = nc.dram_tensor("xsbv", [P, NCH, C], BF16, kind="ExternalInput")
    out_d = nc.dram_tensor("out", [P, NCH, C], BF16, kind="ExternalOutput")

    with tile.TileContext(nc) as tc:
        with (
            tc.tile_pool(name="consts", bufs=1) as consts,
            tc.tile_pool(name="e2pool", bufs=8) as e2pool,
            tc.tile_pool(name="rpool", bufs=4) as rpool,
            tc.tile_pool(name="ps_st", bufs=2, space="PSUM") as ps_st,
            tc.tile_pool(name="ps_v", bufs=1, space="PSUM") as ps_v,
            tc.tile_pool(name="ps_o", bufs=2, space="PSUM") as ps_o,
            nc.allow_low_precision(reason="fp8/bf16 kernel by design"),
        ):
            # ---- input DMAs, in critical-path order ----
            g8 = consts.tile([P, 2, C], F8, tag="g8", name="g8")
            nc.sync.dma_start(out=g8, in_=g8_d[:, :, :])
            xs8 = consts.tile([P, 2, N], F8, tag="xs8", name="xs8")
            nc.sync.dma_start(out=xs8, in_=xs8_d[:, :, :])
            xt8 = []
            for t in range(T):
                xt = consts.tile([P, 2, N], F8, tag=f"xt{t}", name=f"xt{t}")
                xt8.append(xt)
            nc.sync.dma_start(out=xt8[0], in_=xt8_d[0])
            wv8 = consts.tile([P, 2, C], F8, tag="wv8", name="wv8")
            nc.sync.dma_start(out=wv8, in_=wv8_d[:, :, :])
            u8 = consts.tile([P, 2, 1], F8, tag="u8", name="u8")
            nc.sync.dma_start(out=u8, in_=u8_d[:, :, :])
            nc.sync.dma_start(out=xt8[1], in_=xt8_d[1])
            nc.sync.dma_start(out=xt8[2], in_=xt8_d[2])
            xsbv = consts.tile([P, NCH, C], BF16, tag="xsbv", name="xsbv")
            nc.sync.dma_start(out=xsbv, in_=xsbv_d[:, :, :])

            # V^T tiles (fp8) with the 48.0 normalizer column
            v2 = [
                [
                    consts.tile([P, 2, C + 1], F8, tag=f"v2_{t}_{pr}",
                                name=f"v2_{t}_{pr}")
                    for pr in range(PAIRS)
                ]
                for t in range(T)
            ]
            for t in range(T):
                for pr in range(PAIRS):
                    nc.gpsimd.memset(v2[t][pr][:, :, C:C + 1], ZCOL)

            acc = consts.tile([P, NCH, C], BF16, tag="acc", name="acc")

            # ---- P16 = G16^T xs  -> fp8 [128, 2, N] ----
            p8 = consts.tile([P, 2, N], F8, tag="p8", name="p8")
            for bc in range(2):
                pp = ps_st.tile([P, N], F32, tag="st", name=f"pp{bc}")
                for nh in range(2):
                    nc.tensor.matmul(
                        pp[:, nh * 512:(nh + 1) * 512],
                        g8[:, :, bc * P:(bc + 1) * P],
                        xs8[:, :, nh * 512:(nh + 1) * 512],
                        start=True, stop=True, perf_mode=DR,
                    )
                for nh in range(2):
                    nc.scalar.copy(
                        p8[:, bc, nh * 512:(nh + 1) * 512],
                        pp[:, nh * 512:(nh + 1) * 512],
                    )

            kb_sb = [None] * T
            e2 = [[None] * PAIRS for _ in range(T)]

            def emit_kb(t):
                kbp = ps_v.tile([P, MC], F32, tag="kbp", name=f"kbp{t}")
                for mi in range(MC):
                    nc.tensor.matmul(
                        kbp[:, mi:mi + 1],
                        xt8[t][:, :, mi * P:(mi + 1) * P],
                        u8,
                        start=True, stop=True, perf_mode=DR,
                    )
                kb = consts.tile([P, MC], F32, tag=f"kb{t}", name=f"kb{t}")
                nc.vector.tensor_scalar(
                    kb, kbp, SCALE, EXP_SHIFT, AOT.mult, AOT.add
                )
                kb_sb[t] = kb

            def emit_v(t):
                for pr in range(PAIRS):
                    vp = ps_v.tile([P, 2 * C], F32, tag="vp", name=f"vp{t}{pr}")
                    for j in range(2):
                        mi = 2 * pr + j
                        nc.tensor.matmul(
                            vp[:, j * C:(j + 1) * C],
                            xt8[t][:, :, mi * P:(mi + 1) * P],
                            wv8,
                            start=True, stop=True, perf_mode=DR,
                        )
                    for j in range(2):
                        nc.vector.tensor_copy(
                            v2[t][pr][:, j, 0:C], vp[:, j * C:(j + 1) * C]
                        )

            def emit_s(t, mi_list):
                for mi in mi_list:
                    st = ps_st.tile([P, N], F32, tag="st", name=f"st{t}_{mi}")
                    for nh in range(2):
                        nc.tensor.matmul(
                            st[:, nh * 512:(nh + 1) * 512],
                            xt8[t][:, :, mi * P:(mi + 1) * P],
                            p8[:, :, nh * 512:(nh + 1) * 512],
                            start=True, stop=True, perf_mode=DR,
                        )
                    pr, j = mi // 2, mi % 2
                    if e2[t][pr] is None:
                        e2[t][pr] = e2pool.tile([P, 2, N], F8E5, tag="e2",
                                                name=f"e{t}_{pr}")
                    nc.scalar.activation(
                        e2[t][pr][:, j, :],
                        st,
                        func=mybir.ActivationFunctionType.Exp,
                        bias=kb_sb[t][:, mi:mi + 1],
                        scale=SCALE,
                    )

            def emit_o(t):
                for ncx in range(NCH):
                    otp = ps_o.tile([P, C + 1], F32, tag="otp",
                                    name=f"o{t}_{ncx}")
                    for pr in range(PAIRS):
                        nc.tensor.matmul(
                            otp,
                            e2[t][pr][:, :, ncx * P:(ncx + 1) * P],
                            v2[t][pr],
                            start=(pr == 0), stop=(pr == PAIRS - 1),
                            perf_mode=DR,
                        )
                    r = rpool.tile([P, 1], F32, tag="r", name=f"r{t}_{ncx}")
                    nc.vector.reciprocal(r, otp[:, C:C + 1])
                    nc.vector.scalar_tensor_tensor(
                        out=acc[:, ncx, :],
                        in0=otp[:, 0:C],
                        scalar=r,
                        in1=(xsbv[:, ncx, :] if t == 0 else acc[:, ncx, :]),
                        op0=AOT.mult,
                        op1=AOT.add,
                    )

            # ---- software-pipelined teacher loop ----
            emit_kb(0)
            emit_v(0)
            emit_s(0, range(MC))
            for t in range(T):
                if t + 1 < T:
                    emit_kb(t + 1)
                    emit_v(t + 1)
                    emit_s(t + 1, [0, 1])
                emit_o(t)
                if t + 1 < T:
                    emit_s(t + 1, range(2, MC))

            # ---- store (split for earlier start) ----
            nc.sync.dma_start(out=out_d[:, 0:4, :], in_=acc[:, 0:4, :])
            nc.sync.dma_start(out=out_d[:, 4:8, :], in_=acc[:, 4:8, :])

    _split_multi_waits(nc)
    if not nc.is_finalized():
        nc.finalize()
    return nc


def _split_multi_waits(nc):
    """walrus can encode at most one sync-wait per instruction. Hoist every
    wait of a multi-wait instruction onto single-wait nops on the same
    engine, placed immediately before it in program order."""
    fixes = []
    for fn in nc.m.functions:
        for blk in fn.blocks:
            for inst in blk.instructions:
                si = getattr(inst, "sync_info", None)
                if (si is not None and si.on_wait and len(si.on_wait) > 1
                        and getattr(inst, "engine", None) is not None):
                    fixes.append((blk, inst))
    for blk, inst in fixes:
        si = inst.sync_info
        waits = list(si.on_wait)
        nops = []
        for w in waits:
            nop = nc.engines[inst.engine].nop(nofuse=True).ins
            nop.sync_info = mybir.SyncInfo(on_wait=[w], on_update=[])
            nops.append(nop)
        inst.sync_info = mybir.SyncInfo(on_wait=[], on_update=list(si.on_update))
        nop_names = {n.name for n in nops}
        for fn2 in nc.m.functions:
            for blk2 in fn2.blocks:
                blk2.instructions = [
                    i for i in blk2.instructions if i.name not in nop_names
                ]
        pos = next(i for i, x in enumerate(blk.instructions)
                   if x.name == inst.name)
        blk.instructions = (blk.instructions[:pos] + nops
                            + blk.instructions[pos:])


_NC = None


def _get_nc():
    global _NC
    if _NC is None:
        _NC = build_nc()
    return _NC


FP8 = ml_dtypes.float8_e4m3
NPBF16 = ml_dtypes.bfloat16


def _pair2(a):
    """[256, X...] -> [128, 2, X...] with rows (k, k+128) paired."""
    return np.ascontiguousarray(
        a.reshape(2, P, *a.shape[1:]).swapaxes(0, 1)
    )


def make_in_maps(student_feat, t_feat0, t_feat1, t_feat2,
                 Wq, bq, Wk, bk, Wv, bv):
    del bk  # constant per softmax column -> cancels
    Wq = np.asarray(Wq, np.float64)
    Wk = np.asarray(Wk, np.float64)
    Wv = np.asarray(Wv, np.float64)
    bq = np.asarray(bq, np.float64)
    bv = np.asarray(bv, np.float32)
    g16 = (16.0 * (Wq.T @ Wk)).astype(np.float32)
    u16 = (16.0 * (Wk.T @ bq)).astype(np.float32)
    wvT16 = (16.0 * Wv.T).astype(np.float32)

    g8 = _pair2(g16).astype(FP8)
    wv8 = _pair2(wvT16).astype(FP8)
    u8 = _pair2(u16.reshape(C, 1)).astype(FP8)

    xs = np.asarray(student_feat, np.float32).reshape(B, C, N)
    xt = np.stack(
        [np.asarray(t, np.float32) for t in (t_feat0, t_feat1, t_feat2)],
        axis=1,
    ).reshape(B, T, C, N)

    in_maps = []
    for b in range(B):
        xs8 = _pair2(xs[b]).astype(FP8)
        xt8 = np.stack([_pair2(xt[b, t]) for t in range(T)]).astype(FP8)
        xsbv = np.ascontiguousarray(
            (xs[b].T + bv[None, :]).reshape(NCH, P, C).swapaxes(0, 1)
        ).astype(NPBF16)
        in_maps.append({
            "xs8": xs8, "g8": g8, "xt8": xt8, "wv8": wv8, "u8": u8,
            "xsbv": xsbv,
        })
    return in_maps


def run(in_maps, trace=False):
    nc = _get_nc()
    return run_bass_kernel_spmd(nc, in_maps, core_ids=list(range(B)),
                                trace=trace)


def gather_out(res):
    outs = []
    for b in range(B):
        o = np.asarray(res.results[b]["out"], np.float32)  # [128, 8, 256]
        o = o.swapaxes(0, 1).reshape(N, C)  # [n, c]
        outs.append(o.T.reshape(C, H, W))
    return np.stack(outs)


def kernel(student_feat, t_feat0, t_feat1, t_feat2,
           Wq, bq, Wk, bk, Wv, bv):
    in_maps = make_in_maps(student_feat, t_feat0, t_feat1, t_feat2,
                           Wq, bq, Wk, bk, Wv, bv)
    res = run(in_maps, trace=False)
    return gather_out(res).astype(np.float32)


# revision 11
# speedup vs baseline: 2.4273x; 1.0896x over previous
"""CrossTeacherAttention Trainium2 kernel (v2).

Math (per batch element b; xs/xt as [C=256, N=1024], N=H*W):
  S_t[n,m] = scale * (q_n . k_m) with q = Wq xs + bq, k_t = Wk xt_t + bk.
  bk-terms are constant per softmax column -> dropped (softmax invariant).
  S_t = xs^T (Wq^T Wk) xt_t + (Wk^T bq)^T xt_t.  Host folds the weights:
    G16 = 16 Wq^T Wk,  u16 = 16 Wk^T bq  (x16 keeps fp8 entries in normal
    range), so with P16 = G16^T xs the exp argument is
    (xt^T P16)[m,n]/256 + kb_t[m]  with kb_t = (xt_t^T u16)/256 - 2
  (-2 is a uniform shift so exp output fits fp8; cancels in softmax).
  attn.mean(-1) of a softmax is exactly 1/N -> teacher weights are 1/3.
  out = xs + bv + sum_t (E'_t V_t) / (3 Z'_t),  Z'_t[n] = sum_m E'_t[m,n].

Layout trick: O is computed with E' as the *stationary* matmul operand
(lhsT = E'[m,2,n], rhs = V^T[m,2,c]) so the output lands as [n_part, c_free],
where the softmax normalizer is a per-partition scalar.  V^T gets a 257th
column holding 48.0, so column 256 of the O accumulator is 48*Z'_t and
(1/48)*16(V-scale) = 1/3 folds the teacher weight into the reciprocal.
Z row-sums and the bias fold thus cost no extra PE streaming.

All matmuls run in fp8e4 DoubleRow (two 128-row k-tiles per pass).  exp runs
on ACT reading [128,1024] PSUM spans, writing fp8 E' directly.  Inputs are
pre-quantized/interleaved on the host ([128, 2, *] k-pair layout), which also
halves HBM traffic.  Residual path (xs^T + bv) ships as bf16; output is bf16
[n, c], transposed back on the host.

Sharding: data-parallel over batch, B=8 -> one batch element per core.
"""

import sys

sys.path.insert(0, "/opt/trn_rl_repo")

import ml_dtypes
import numpy as np

import concourse.bass as bass
import concourse.tile as tile
from concourse import mybir
from concourse.bass_utils import run_bass_kernel_spmd

B, C, H, W = 8, 256, 32, 32
N = H * W  # 1024
T = 3
P = 128
NCH = N // P  # 8 n-chunks
MC = N // P  # 8 m-chunks
PAIRS = MC // 2  # 4 m-chunk pairs (DoubleRow)
F32 = mybir.dt.float32
BF16 = mybir.dt.bfloat16
F8 = mybir.dt.float8e4
F8E5 = mybir.dt.float8e5
DR = mybir.MatmulPerfMode.DoubleRow
SCALE = 1.0 / 256.0  # 1/sqrt(C) / 16 (the x16 weight prescale)
EXP_SHIFT = -2.0
ZCOL = 48.0  # 16 (V prescale) * 3 (teacher weight)
AOT = mybir.AluOpType


def build_nc():
    nc = bass.Bass()
    xs8_d = nc.dram_tensor("xs8", [P, 2, N], F8, kind="ExternalInput")
    g8_d = nc.dram_tensor("g8", [P, 2, C], F8, kind="ExternalInput")
    xt8_d = nc.dram_tensor("xt8", [T, P, 2, N], F8, kind="ExternalInput")
    wv8_d = nc.dram_tensor("wv8", [P, 2, C], F8, kind="ExternalInput")
    kb_d = nc.dram_tensor("kb", [P, T * MC], F32, kind="ExternalInput")
    xsbv_d = nc.dram_tensor("xsbv", [P, NCH, C], BF16, kind="ExternalInput")
    out_d = nc.dram_tensor("out", [P, NCH, C], BF16, kind="ExternalOutput")

    with tile.TileContext(nc) as tc:
        with (
            tc.tile_pool(name="consts", bufs=1) as consts,
            tc.tile_pool(name="e2pool", bufs=8) as e2pool,
            tc.tile_pool(name="rpool", bufs=4) as rpool,
            tc.tile_pool(name="ps_st", bufs=2, space="PSUM") as ps_st,
            tc.tile_pool(name="ps_v", bufs=1, space="PSUM") as ps_v,
            tc.tile_pool(name="ps_o", bufs=3, space="PSUM") as ps_o,
            nc.allow_low_precision(reason="fp8/bf16 kernel by design"),
        ):
            # ---- ACT table warmup: dummy exp so the exp/copy table
            # load happens during the input-DMA wait, off the critical path
            warm = consts.tile([P, 1], F32, tag="warm", name="warm")
            nc.gpsimd.memset(warm, 0.0)
            warm_o = consts.tile([P, 1], F32, tag="warm_o", name="warm_o")
            nc.scalar.activation(
                warm_o, warm, func=mybir.ActivationFunctionType.Exp, scale=0.0
            )

            # ---- input DMAs, in critical-path order ----
            g8 = consts.tile([P, 2, C], F8, tag="g8", name="g8")
            nc.sync.dma_start(out=g8, in_=g8_d[:, :, :])
            xs8 = consts.tile([P, 2, N], F8, tag="xs8", name="xs8")
            nc.sync.dma_start(out=xs8, in_=xs8_d[:, :, :])
            kb_sb = consts.tile([P, T * MC], F32, tag="kb", name="kb")
            nc.sync.dma_start(out=kb_sb, in_=kb_d[:, :])
            xt8 = []
            for t in range(T):
                xt = consts.tile([P, 2, N], F8, tag=f"xt{t}", name=f"xt{t}")
                xt8.append(xt)
            nc.sync.dma_start(out=xt8[0], in_=xt8_d[0])
            wv8 = consts.tile([P, 2, C], F8, tag="wv8", name="wv8")
            nc.sync.dma_start(out=wv8, in_=wv8_d[:, :, :])
            nc.sync.dma_start(out=xt8[1], in_=xt8_d[1])
            nc.sync.dma_start(out=xt8[2], in_=xt8_d[2])
            xsbv = consts.tile([P, NCH, C], BF16, tag="xsbv", name="xsbv")
            nc.sync.dma_start(out=xsbv, in_=xsbv_d[:, :, :])

            # V^T tiles (fp8) with the 48.0 normalizer column
            v2 = [
                [
                    consts.tile([P, 2, C + 1], F8, tag=f"v2_{t}_{pr}",
                                name=f"v2_{t}_{pr}")
                    for pr in range(PAIRS)
                ]
                for t in range(T)
            ]
            for t in range(T):
                for pr in range(PAIRS):
                    nc.gpsimd.memset(v2[t][pr][:, :, C:C + 1], ZCOL)

            acc = consts.tile([P, NCH, C], BF16, tag="acc", name="acc")

            # ---- P16 = G16^T xs  -> fp8 [128, 2, N] ----
            p8 = consts.tile([P, 2, N], F8, tag="p8", name="p8")
            for bc in range(2):
                pp = ps_st.tile([P, N], F32, tag="st", name=f"pp{bc}")
                for nh in range(2):
                    nc.tensor.matmul(
                        pp[:, nh * 512:(nh + 1) * 512],
                        g8[:, :, bc * P:(bc + 1) * P],
                        xs8[:, :, nh * 512:(nh + 1) * 512],
                        start=True, stop=True, perf_mode=DR,
                    )
                for nh in range(2):
                    eng = nc.vector.tensor_copy if bc == 0 else nc.scalar.copy
                    eng(
                        p8[:, bc, nh * 512:(nh + 1) * 512],
                        pp[:, nh * 512:(nh + 1) * 512],
                    )

            e2 = [[None] * PAIRS for _ in range(T)]

            def emit_v(t):
                for pr in range(PAIRS):
                    vp = ps_v.tile([P, 2 * C], F32, tag="vp", name=f"vp{t}{pr}")
                    for j in range(2):
                        mi = 2 * pr + j
                        nc.tensor.matmul(
                            vp[:, j * C:(j + 1) * C],
                            xt8[t][:, :, mi * P:(mi + 1) * P],
                            wv8,
                            start=True, stop=True, perf_mode=DR,
                        )
                    for j in range(2):
                        nc.vector.tensor_copy(
                            v2[t][pr][:, j, 0:C], vp[:, j * C:(j + 1) * C]
                        )

            def emit_s(t, mi_list):
                for mi in mi_list:
                    st = ps_st.tile([P, N], F32, tag="st", name=f"st{t}_{mi}")
                    for nh in range(2):
                        nc.tensor.matmul(
                            st[:, nh * 512:(nh + 1) * 512],
                            xt8[t][:, :, mi * P:(mi + 1) * P],
                            p8[:, :, nh * 512:(nh + 1) * 512],
                            start=True, stop=True, perf_mode=DR,
                        )
                    pr, j = mi // 2, mi % 2
                    if e2[t][pr] is None:
                        e2[t][pr] = e2pool.tile([P, 2, N], F8E5, tag="e2",
                                                name=f"e{t}_{pr}")
                    nc.scalar.activation(
                        e2[t][pr][:, j, :],
                        st,
                        func=mybir.ActivationFunctionType.Exp,
                        bias=kb_sb[:, t * MC + mi:t * MC + mi + 1],
                        scale=SCALE,
                    )

            def emit_o_group(t, ncx, act_assist=False):
                otp = ps_o.tile([P, C + 1], F32, tag="otp",
                                name=f"o{t}_{ncx}")
                for pr in range(PAIRS):
                    nc.tensor.matmul(
                        otp,
                        e2[t][pr][:, :, ncx * P:(ncx + 1) * P],
                        v2[t][pr],
                        start=(pr == 0), stop=(pr == PAIRS - 1),
                        perf_mode=DR,
                    )
                r = rpool.tile([P, 1], F32, tag="r", name=f"r{t}_{ncx}")
                nc.vector.reciprocal(r, otp[:, C:C + 1])
                base = xsbv[:, ncx, :] if t == 0 else acc[:, ncx, :]
                if act_assist:
                    # post-exp tail: ACT is idle, split mul (ACT) + add (DVE)
                    tmp = rpool.tile([P, C], BF16, tag="tmp", name=f"tm{ncx}")
                    nc.scalar.activation(
                        tmp, otp[:, 0:C],
                        func=mybir.ActivationFunctionType.Copy, scale=r,
                    )
                    nc.vector.tensor_add(acc[:, ncx, :], tmp, base)
                else:
                    nc.vector.scalar_tensor_tensor(
                        out=acc[:, ncx, :],
                        in0=otp[:, 0:C],
                        scalar=r,
                        in1=base,
                        op0=AOT.mult,
                        op1=AOT.add,
                    )

            # ---- software-pipelined teacher loop ----
            emit_v(0)
            emit_s(0, range(MC))
            for t in range(T):
                if t + 1 < T:
                    emit_v(t + 1)
                    emit_s(t + 1, [0, 1])
                    for k in range(NCH):
                        if k + 2 < MC:
                            emit_s(t + 1, [k + 2])
                        emit_o_group(t, k)
                else:
                    for k in range(NCH):
                        emit_o_group(t, k, act_assist=(k % 2 == 0))
                        if k % 2 == 1:
                            nc.sync.dma_start(
                                out=out_d[:, k - 1:k + 1, :],
                                in_=acc[:, k - 1:k + 1, :],
                            )

    _split_multi_waits(nc)
    if not nc.is_finalized():
        nc.finalize()
    return nc


def _split_multi_waits(nc):
    """walrus can encode at most one sync-wait per instruction. Hoist every
    wait of a multi-wait instruction onto single-wait nops on the same
    engine, placed immediately before it in program order."""
    fixes = []
    for fn in nc.m.functions:
        for blk in fn.blocks:
            for inst in blk.instructions:
                si = getattr(inst, "sync_info", None)
                if (si is not None and si.on_wait and len(si.on_wait) > 1
                        and getattr(inst, "engine", None) is not None):
                    fixes.append((blk, inst))
    for blk, inst in fixes:
        si = inst.sync_info
        waits = list(si.on_wait)
        nops = []
        for w in waits:
            nop = nc.engines[inst.engine].nop(nofuse=True).ins
            nop.sync_info = mybir.SyncInfo(on_wait=[w], on_update=[])
            nops.append(nop)
        inst.sync_info = mybir.SyncInfo(on_wait=[], on_update=list(si.on_update))
        nop_names = {n.name for n in nops}
        for fn2 in nc.m.functions:
            for blk2 in fn2.blocks:
                blk2.instructions = [
                    i for i in blk2.instructions if i.name not in nop_names
                ]
        pos = next(i for i, x in enumerate(blk.instructions)
                   if x.name == inst.name)
        blk.instructions = (blk.instructions[:pos] + nops
                            + blk.instructions[pos:])


_NC = None


def _get_nc():
    global _NC
    if _NC is None:
        _NC = build_nc()
    return _NC


FP8 = ml_dtypes.float8_e4m3
NPBF16 = ml_dtypes.bfloat16


def _pair2(a):
    """[256, X...] -> [128, 2, X...] with rows (k, k+128) paired."""
    return np.ascontiguousarray(
        a.reshape(2, P, *a.shape[1:]).swapaxes(0, 1)
    )


def make_in_maps(student_feat, t_feat0, t_feat1, t_feat2,
                 Wq, bq, Wk, bk, Wv, bv):
    del bk  # constant per softmax column -> cancels
    Wq = np.asarray(Wq, np.float64)
    Wk = np.asarray(Wk, np.float64)
    Wv = np.asarray(Wv, np.float64)
    bq = np.asarray(bq, np.float64)
    bv = np.asarray(bv, np.float32)
    g16 = (16.0 * (Wq.T @ Wk)).astype(np.float32)
    u16 = (16.0 * (Wk.T @ bq)).astype(np.float32)
    wvT16 = (16.0 * Wv.T).astype(np.float32)

    g8 = _pair2(g16).astype(FP8)
    wv8 = _pair2(wvT16).astype(FP8)

    xs = np.asarray(student_feat, np.float32).reshape(B, C, N)
    xt = np.stack(
        [np.asarray(t, np.float32) for t in (t_feat0, t_feat1, t_feat2)],
        axis=1,
    ).reshape(B, T, C, N)

    in_maps = []
    for b in range(B):
        xs8 = _pair2(xs[b]).astype(FP8)
        xt8 = np.stack([_pair2(xt[b, t]) for t in range(T)]).astype(FP8)
        xsbv = np.ascontiguousarray(
            (xs[b].T + bv[None, :]).reshape(NCH, P, C).swapaxes(0, 1)
        ).astype(NPBF16)
        # per-key bias fold: kb_t[m] = (xt_t[:,m] . Wk^T bq)/16 - 2
        kb = np.empty((P, T * MC), np.float32)
        for t in range(T):
            v = (xt[b, t].T @ (u16 / 256.0)) + EXP_SHIFT  # [N]
            kb[:, t * MC:(t + 1) * MC] = v.reshape(MC, P).T
        in_maps.append({
            "xs8": xs8, "g8": g8, "xt8": xt8, "wv8": wv8,
            "kb": kb, "xsbv": xsbv,
        })
    return in_maps


def run(in_maps, trace=False):
    nc = _get_nc()
    return run_bass_kernel_spmd(nc, in_maps, core_ids=list(range(B)),
                                trace=trace)


def gather_out(res):
    outs = []
    for b in range(B):
        o = np.asarray(res.results[b]["out"], np.float32)  # [128, 8, 256]
        o = o.swapaxes(0, 1).reshape(N, C)  # [n, c]
        outs.append(o.T.reshape(C, H, W))
    return np.stack(outs)


def kernel(student_feat, t_feat0, t_feat1, t_feat2,
           Wq, bq, Wk, bk, Wv, bv):
    in_maps = make_in_maps(student_feat, t_feat0, t_feat1, t_feat2,
                           Wq, bq, Wk, bk, Wv, bv)
    res = run(in_maps, trace=False)
    return gather_out(res).astype(np.float32)


# revision 12
# speedup vs baseline: 2.4281x; 1.0003x over previous
"""CrossTeacherAttention Trainium2 kernel (v2).

Math (per batch element b; xs/xt as [C=256, N=1024], N=H*W):
  S_t[n,m] = scale * (q_n . k_m) with q = Wq xs + bq, k_t = Wk xt_t + bk.
  bk-terms are constant per softmax column -> dropped (softmax invariant).
  S_t = xs^T (Wq^T Wk) xt_t + (Wk^T bq)^T xt_t.  Host folds the weights:
    G16 = 16 Wq^T Wk,  u16 = 16 Wk^T bq  (x16 keeps fp8 entries in normal
    range), so with P16 = G16^T xs the exp argument is
    (xt^T P16)[m,n]/256 + kb_t[m]  with kb_t = (xt_t^T u16)/256 - 2
  (-2 is a uniform shift so exp output fits fp8; cancels in softmax).
  attn.mean(-1) of a softmax is exactly 1/N -> teacher weights are 1/3.
  out = xs + bv + sum_t (E'_t V_t) / (3 Z'_t),  Z'_t[n] = sum_m E'_t[m,n].

Layout trick: O is computed with E' as the *stationary* matmul operand
(lhsT = E'[m,2,n], rhs = V^T[m,2,c]) so the output lands as [n_part, c_free],
where the softmax normalizer is a per-partition scalar.  V^T gets a 257th
column holding 48.0, so column 256 of the O accumulator is 48*Z'_t and
(1/48)*16(V-scale) = 1/3 folds the teacher weight into the reciprocal.
Z row-sums and the bias fold thus cost no extra PE streaming.

All matmuls run in fp8e4 DoubleRow (two 128-row k-tiles per pass).  exp runs
on ACT reading [128,1024] PSUM spans, writing fp8 E' directly.  Inputs are
pre-quantized/interleaved on the host ([128, 2, *] k-pair layout), which also
halves HBM traffic.  Residual path (xs^T + bv) ships as bf16; output is bf16
[n, c], transposed back on the host.

Sharding: data-parallel over batch, B=8 -> one batch element per core.
"""

import sys

sys.path.insert(0, "/opt/trn_rl_repo")

import ml_dtypes
import numpy as np

import concourse.bass as bass
import concourse.tile as tile
from concourse import mybir
from concourse.bass_utils import run_bass_kernel_spmd

B, C, H, W = 8, 256, 32, 32
N = H * W  # 1024
T = 3
P = 128
NCH = N // P  # 8 n-chunks
MC = N // P  # 8 m-chunks
PAIRS = MC // 2  # 4 m-chunk pairs (DoubleRow)
F32 = mybir.dt.float32
BF16 = mybir.dt.bfloat16
F8 = mybir.dt.float8e4
F8E5 = mybir.dt.float8e5
DR = mybir.MatmulPerfMode.DoubleRow
SCALE = 1.0 / 256.0  # 1/sqrt(C) / 16 (the x16 weight prescale)
EXP_SHIFT = -2.0
ZCOL = 48.0  # 16 (V prescale) * 3 (teacher weight)
AOT = mybir.AluOpType


def build_nc():
    nc = bass.Bass()
    px8_d = nc.dram_tensor("px8", [P, 2, C + N], F8, kind="ExternalInput")
    xt8_d = nc.dram_tensor("xt8", [T, P, 2, N], F8, kind="ExternalInput")
    wv8_d = nc.dram_tensor("wv8", [P, 2, C], F8, kind="ExternalInput")
    kb_d = nc.dram_tensor("kb", [P, T * MC], F32, kind="ExternalInput")
    xsbv_d = nc.dram_tensor("xsbv", [P, NCH, C], BF16, kind="ExternalInput")
    out_d = nc.dram_tensor("out", [P, NCH, C], BF16, kind="ExternalOutput")

    with tile.TileContext(nc) as tc:
        with (
            tc.tile_pool(name="consts", bufs=1) as consts,
            tc.tile_pool(name="e2pool", bufs=8) as e2pool,
            tc.tile_pool(name="rpool", bufs=4) as rpool,
            tc.tile_pool(name="ps_st", bufs=2, space="PSUM") as ps_st,
            tc.tile_pool(name="ps_v", bufs=1, space="PSUM") as ps_v,
            tc.tile_pool(name="ps_o", bufs=3, space="PSUM") as ps_o,
            nc.allow_low_precision(reason="fp8/bf16 kernel by design"),
        ):
            # ---- ACT table warmup: dummy exp so the exp/copy table
            # load happens during the input-DMA wait, off the critical path
            warm = consts.tile([P, 1], F32, tag="warm", name="warm")
            nc.gpsimd.memset(warm, 0.0)
            warm_o = consts.tile([P, 1], F32, tag="warm_o", name="warm_o")
            nc.scalar.activation(
                warm_o, warm, func=mybir.ActivationFunctionType.Exp, scale=0.0
            )

            # ---- input DMAs, in critical-path order ----
            px8 = consts.tile([P, 2, C + N], F8, tag="px8", name="px8")
            nc.sync.dma_start(out=px8, in_=px8_d[:, :, :])
            kb_sb = consts.tile([P, T * MC], F32, tag="kb", name="kb")
            nc.sync.dma_start(out=kb_sb, in_=kb_d[:, :])
            xt8 = []
            for t in range(T):
                xt = consts.tile([P, 2, N], F8, tag=f"xt{t}", name=f"xt{t}")
                xt8.append(xt)
            nc.sync.dma_start(out=xt8[0], in_=xt8_d[0])
            wv8 = consts.tile([P, 2, C], F8, tag="wv8", name="wv8")
            nc.sync.dma_start(out=wv8, in_=wv8_d[:, :, :])
            nc.sync.dma_start(out=xt8[1], in_=xt8_d[1])
            nc.sync.dma_start(out=xt8[2], in_=xt8_d[2])
            xsbv = consts.tile([P, NCH, C], BF16, tag="xsbv", name="xsbv")
            nc.sync.dma_start(out=xsbv, in_=xsbv_d[:, :, :])

            # V^T tiles (fp8) with the 48.0 normalizer column
            v2 = [
                [
                    consts.tile([P, 2, C + 1], F8, tag=f"v2_{t}_{pr}",
                                name=f"v2_{t}_{pr}")
                    for pr in range(PAIRS)
                ]
                for t in range(T)
            ]
            for t in range(T):
                for pr in range(PAIRS):
                    nc.gpsimd.memset(v2[t][pr][:, :, C:C + 1], ZCOL)

            acc = consts.tile([P, NCH, C], BF16, tag="acc", name="acc")

            # ---- P16 = G16^T xs  -> fp8 [128, 2, N] ----
            p8 = consts.tile([P, 2, N], F8, tag="p8", name="p8")
            for bc in range(2):
                pp = ps_st.tile([P, N], F32, tag="st", name=f"pp{bc}")
                for nh in range(2):
                    nc.tensor.matmul(
                        pp[:, nh * 512:(nh + 1) * 512],
                        px8[:, :, bc * P:(bc + 1) * P],
                        px8[:, :, C + nh * 512:C + (nh + 1) * 512],
                        start=True, stop=True, perf_mode=DR,
                    )
                for nh in range(2):
                    eng = nc.vector.tensor_copy if bc == 0 else nc.scalar.copy
                    eng(
                        p8[:, bc, nh * 512:(nh + 1) * 512],
                        pp[:, nh * 512:(nh + 1) * 512],
                    )

            e2 = [[None] * PAIRS for _ in range(T)]

            def emit_v(t):
                for pr in range(PAIRS):
                    vp = ps_v.tile([P, 2 * C], F32, tag="vp", name=f"vp{t}{pr}")
                    for j in range(2):
                        mi = 2 * pr + j
                        nc.tensor.matmul(
                            vp[:, j * C:(j + 1) * C],
                            xt8[t][:, :, mi * P:(mi + 1) * P],
                            wv8,
                            start=True, stop=True, perf_mode=DR,
                        )
                    for j in range(2):
                        nc.vector.tensor_copy(
                            v2[t][pr][:, j, 0:C], vp[:, j * C:(j + 1) * C]
                        )

            def emit_s(t, mi_list):
                for mi in mi_list:
                    st = ps_st.tile([P, N], F32, tag="st", name=f"st{t}_{mi}")
                    for nh in range(2):
                        nc.tensor.matmul(
                            st[:, nh * 512:(nh + 1) * 512],
                            xt8[t][:, :, mi * P:(mi + 1) * P],
                            p8[:, :, nh * 512:(nh + 1) * 512],
                            start=True, stop=True, perf_mode=DR,
                        )
                    pr, j = mi // 2, mi % 2
                    if e2[t][pr] is None:
                        e2[t][pr] = e2pool.tile([P, 2, N], F8E5, tag="e2",
                                                name=f"e{t}_{pr}")
                    nc.scalar.activation(
                        e2[t][pr][:, j, :],
                        st,
                        func=mybir.ActivationFunctionType.Exp,
                        bias=kb_sb[:, t * MC + mi:t * MC + mi + 1],
                        scale=SCALE,
                    )

            def emit_o_group(t, ncx, act_assist=False, gps_add=False):
                otp = ps_o.tile([P, C + 1], F32, tag="otp",
                                name=f"o{t}_{ncx}")
                for pr in range(PAIRS):
                    nc.tensor.matmul(
                        otp,
                        e2[t][pr][:, :, ncx * P:(ncx + 1) * P],
                        v2[t][pr],
                        start=(pr == 0), stop=(pr == PAIRS - 1),
                        perf_mode=DR,
                    )
                r = rpool.tile([P, 1], F32, tag="r", name=f"r{t}_{ncx}")
                nc.vector.reciprocal(r, otp[:, C:C + 1])
                base = xsbv[:, ncx, :] if t == 0 else acc[:, ncx, :]
                if act_assist:
                    # post-exp tail: ACT is idle, split mul (ACT) + add
                    tmp = rpool.tile([P, C], BF16, tag="tmp", name=f"tm{ncx}")
                    nc.scalar.activation(
                        tmp, otp[:, 0:C],
                        func=mybir.ActivationFunctionType.Copy, scale=r,
                    )
                    add_eng = nc.gpsimd if gps_add else nc.vector
                    add_eng.tensor_add(acc[:, ncx, :], tmp, base)
                else:
                    nc.vector.scalar_tensor_tensor(
                        out=acc[:, ncx, :],
                        in0=otp[:, 0:C],
                        scalar=r,
                        in1=base,
                        op0=AOT.mult,
                        op1=AOT.add,
                    )

            # ---- software-pipelined teacher loop ----
            emit_v(0)
            emit_s(0, range(MC))
            for t in range(T):
                if t + 1 < T:
                    emit_v(t + 1)
                    emit_s(t + 1, [0, 1])
                    for k in range(NCH):
                        if k + 2 < MC:
                            emit_s(t + 1, [k + 2])
                        emit_o_group(t, k)
                else:
                    for k in range(NCH):
                        emit_o_group(t, k, act_assist=(k % 2 == 0),
                                     gps_add=(k in (0, 2)))
                        if k in (1, 3, 5):
                            nc.sync.dma_start(
                                out=out_d[:, k - 1:k + 1, :],
                                in_=acc[:, k - 1:k + 1, :],
                            )
                        elif k >= 6:
                            nc.sync.dma_start(
                                out=out_d[:, k:k + 1, :],
                                in_=acc[:, k:k + 1, :],
                            )

    _split_multi_waits(nc)
    if not nc.is_finalized():
        nc.finalize()
    return nc


def _split_multi_waits(nc):
    """walrus can encode at most one sync-wait per instruction. Hoist every
    wait of a multi-wait instruction onto single-wait nops on the same
    engine, placed immediately before it in program order."""
    fixes = []
    for fn in nc.m.functions:
        for blk in fn.blocks:
            for inst in blk.instructions:
                si = getattr(inst, "sync_info", None)
                if (si is not None and si.on_wait and len(si.on_wait) > 1
                        and getattr(inst, "engine", None) is not None):
                    fixes.append((blk, inst))
    for blk, inst in fixes:
        si = inst.sync_info
        waits = list(si.on_wait)
        nops = []
        for w in waits:
            nop = nc.engines[inst.engine].nop(nofuse=True).ins
            nop.sync_info = mybir.SyncInfo(on_wait=[w], on_update=[])
            nops.append(nop)
        inst.sync_info = mybir.SyncInfo(on_wait=[], on_update=list(si.on_update))
        nop_names = {n.name for n in nops}
        for fn2 in nc.m.functions:
            for blk2 in fn2.blocks:
                blk2.instructions = [
                    i for i in blk2.instructions if i.name not in nop_names
                ]
        pos = next(i for i, x in enumerate(blk.instructions)
                   if x.name == inst.name)
        blk.instructions = (blk.instructions[:pos] + nops
                            + blk.instructions[pos:])


_NC = None


def _get_nc():
    global _NC
    if _NC is None:
        _NC = build_nc()
    return _NC


FP8 = ml_dtypes.float8_e4m3
NPBF16 = ml_dtypes.bfloat16


def _pair2(a):
    """[256, X...] -> [128, 2, X...] with rows (k, k+128) paired."""
    return np.ascontiguousarray(
        a.reshape(2, P, *a.shape[1:]).swapaxes(0, 1)
    )


def make_in_maps(student_feat, t_feat0, t_feat1, t_feat2,
                 Wq, bq, Wk, bk, Wv, bv):
    del bk  # constant per softmax column -> cancels
    Wq = np.asarray(Wq, np.float64)
    Wk = np.asarray(Wk, np.float64)
    Wv = np.asarray(Wv, np.float64)
    bq = np.asarray(bq, np.float64)
    bv = np.asarray(bv, np.float32)
    g16 = (16.0 * (Wq.T @ Wk)).astype(np.float32)
    u16 = (16.0 * (Wk.T @ bq)).astype(np.float32)
    wvT16 = (16.0 * Wv.T).astype(np.float32)

    g8 = _pair2(g16).astype(FP8)
    wv8 = _pair2(wvT16).astype(FP8)

    xs = np.asarray(student_feat, np.float32).reshape(B, C, N)
    xt = np.stack(
        [np.asarray(t, np.float32) for t in (t_feat0, t_feat1, t_feat2)],
        axis=1,
    ).reshape(B, T, C, N)

    in_maps = []
    for b in range(B):
        px8 = np.concatenate([g8, _pair2(xs[b]).astype(FP8)], axis=2)
        xt8 = np.stack([_pair2(xt[b, t]) for t in range(T)]).astype(FP8)
        xsbv = np.ascontiguousarray(
            (xs[b].T + bv[None, :]).reshape(NCH, P, C).swapaxes(0, 1)
        ).astype(NPBF16)
        # per-key bias fold: kb_t[m] = (xt_t[:,m] . Wk^T bq)/16 - 2
        kb = np.empty((P, T * MC), np.float32)
        for t in range(T):
            v = (xt[b, t].T @ (u16 / 256.0)) + EXP_SHIFT  # [N]
            kb[:, t * MC:(t + 1) * MC] = v.reshape(MC, P).T
        in_maps.append({
            "px8": px8, "xt8": xt8, "wv8": wv8,
            "kb": kb, "xsbv": xsbv,
        })
    return in_maps


def run(in_maps, trace=False):
    nc = _get_nc()
    return run_bass_kernel_spmd(nc, in_maps, core_ids=list(range(B)),
                                trace=trace)


def gather_out(res):
    outs = []
    for b in range(B):
        o = np.asarray(res.results[b]["out"], np.float32)  # [128, 8, 256]
        o = o.swapaxes(0, 1).reshape(N, C)  # [n, c]
        outs.append(o.T.reshape(C, H, W))
    return np.stack(outs)


def kernel(student_feat, t_feat0, t_feat1, t_feat2,
           Wq, bq, Wk, bk, Wv, bv):
    in_maps = make_in_maps(student_feat, t_feat0, t_feat1, t_feat2,
                           Wq, bq, Wk, bk, Wv, bv)
    res = run(in_maps, trace=False)
    return gather_out(res).astype(np.float32)


# revision 13
# speedup vs baseline: 2.5108x; 1.0341x over previous
"""CrossTeacherAttention Trainium2 kernel (v2).

Math (per batch element b; xs/xt as [C=256, N=1024], N=H*W):
  S_t[n,m] = scale * (q_n . k_m) with q = Wq xs + bq, k_t = Wk xt_t + bk.
  bk-terms are constant per softmax column -> dropped (softmax invariant).
  S_t = xs^T (Wq^T Wk) xt_t + (Wk^T bq)^T xt_t.  Host folds the weights:
    G16 = 16 Wq^T Wk,  u16 = 16 Wk^T bq  (x16 keeps fp8 entries in normal
    range), so with P16 = G16^T xs the exp argument is
    (xt^T P16)[m,n]/256 + kb_t[m]  with kb_t = (xt_t^T u16)/256 - 2
  (-2 is a uniform shift so exp output fits fp8; cancels in softmax).
  attn.mean(-1) of a softmax is exactly 1/N -> teacher weights are 1/3.
  out = xs + bv + sum_t (E'_t V_t) / (3 Z'_t),  Z'_t[n] = sum_m E'_t[m,n].

Layout trick: O is computed with E' as the *stationary* matmul operand
(lhsT = E'[m,2,n], rhs = V^T[m,2,c]) so the output lands as [n_part, c_free],
where the softmax normalizer is a per-partition scalar.  V^T gets a 257th
column holding 48.0, so column 256 of the O accumulator is 48*Z'_t and
(1/48)*16(V-scale) = 1/3 folds the teacher weight into the reciprocal.
Z row-sums and the bias fold thus cost no extra PE streaming.

All matmuls run in fp8e4 DoubleRow (two 128-row k-tiles per pass).  exp runs
on ACT reading [128,1024] PSUM spans, writing fp8 E' directly.  Inputs are
pre-quantized/interleaved on the host ([128, 2, *] k-pair layout), which also
halves HBM traffic.  Residual path (xs^T + bv) ships as bf16; output is bf16
[n, c], transposed back on the host.

Sharding: data-parallel over batch, B=8 -> one batch element per core.
"""

import sys

sys.path.insert(0, "/opt/trn_rl_repo")

import ml_dtypes
import numpy as np

import concourse.bass as bass
import concourse.tile as tile
from concourse import mybir
from concourse.bass_utils import run_bass_kernel_spmd

B, C, H, W = 8, 256, 32, 32
N = H * W  # 1024
T = 3
P = 128
NCH = N // P  # 8 n-chunks
MC = N // P  # 8 m-chunks
PAIRS = MC // 2  # 4 m-chunk pairs (DoubleRow)
F32 = mybir.dt.float32
BF16 = mybir.dt.bfloat16
F8 = mybir.dt.float8e4
F8E5 = mybir.dt.float8e5
DR = mybir.MatmulPerfMode.DoubleRow
SCALE = 1.0 / 256.0  # 1/sqrt(C) / 16 (the x16 weight prescale)
EXP_SHIFT = -2.0
ZCOL = 48.0  # 16 (V prescale) * 3 (teacher weight)
L2E4 = 4.0 * 1.4426950408889634  # 4*log2(e): e5m2 exponent scale
K1 = L2E4 / 256.0
KB4_OFF = 60.0 + 0.67  # (15+bias)<<2 plus rounding/curvature compensation
# (t, mi) exp tiles computed on DVE via the e5m2 bit-trick instead of ACT
OFFLOAD = {(0, 6), (2, 2), (2, 5)}
AOT = mybir.AluOpType


def build_nc():
    nc = bass.Bass()
    px8_d = nc.dram_tensor("px8", [P, 2, C + N], F8, kind="ExternalInput")
    xt8_d = nc.dram_tensor("xt8", [T, P, 2, N], F8, kind="ExternalInput")
    wv8_d = nc.dram_tensor("wv8", [P, 2, C], F8, kind="ExternalInput")
    kb_d = nc.dram_tensor("kb", [P, 2 * T * MC], F32, kind="ExternalInput")
    xsbv_d = nc.dram_tensor("xsbv", [P, NCH, C], BF16, kind="ExternalInput")
    out_d = nc.dram_tensor("out", [P, NCH, C], BF16, kind="ExternalOutput")

    with tile.TileContext(nc) as tc:
        with (
            tc.tile_pool(name="consts", bufs=1) as consts,
            tc.tile_pool(name="e2pool", bufs=8) as e2pool,
            tc.tile_pool(name="rpool", bufs=4) as rpool,
            tc.tile_pool(name="ypool", bufs=2) as ypool,
            tc.tile_pool(name="ps_st", bufs=2, space="PSUM") as ps_st,
            tc.tile_pool(name="ps_v", bufs=1, space="PSUM") as ps_v,
            tc.tile_pool(name="ps_o", bufs=3, space="PSUM") as ps_o,
            nc.allow_low_precision(reason="fp8/bf16 kernel by design"),
        ):
            # ---- ACT table warmup: dummy exp so the exp/copy table
            # load happens during the input-DMA wait, off the critical path
            warm = consts.tile([P, 1], F32, tag="warm", name="warm")
            nc.gpsimd.memset(warm, 0.0)
            warm_o = consts.tile([P, 1], F32, tag="warm_o", name="warm_o")
            nc.scalar.activation(
                warm_o, warm, func=mybir.ActivationFunctionType.Exp, scale=0.0
            )

            # ---- input DMAs, in critical-path order ----
            px8 = consts.tile([P, 2, C + N], F8, tag="px8", name="px8")
            nc.sync.dma_start(out=px8[:, :, 0:C + 512],
                              in_=px8_d[:, :, 0:C + 512])
            nc.sync.dma_start(out=px8[:, :, C + 512:],
                              in_=px8_d[:, :, C + 512:])
            kb_sb = consts.tile([P, 2 * T * MC], F32, tag="kb", name="kb")
            nc.sync.dma_start(out=kb_sb, in_=kb_d[:, :])
            xt8 = []
            for t in range(T):
                xt = consts.tile([P, 2, N], F8, tag=f"xt{t}", name=f"xt{t}")
                xt8.append(xt)
            nc.sync.dma_start(out=xt8[0], in_=xt8_d[0])
            wv8 = consts.tile([P, 2, C], F8, tag="wv8", name="wv8")
            nc.sync.dma_start(out=wv8, in_=wv8_d[:, :, :])
            nc.sync.dma_start(out=xt8[1], in_=xt8_d[1])
            nc.sync.dma_start(out=xt8[2], in_=xt8_d[2])
            xsbv = consts.tile([P, NCH, C], BF16, tag="xsbv", name="xsbv")
            nc.sync.dma_start(out=xsbv, in_=xsbv_d[:, :, :])

            # V^T tiles (fp8) with the 48.0 normalizer column
            v2 = [
                [
                    consts.tile([P, 2, C + 1], F8, tag=f"v2_{t}_{pr}",
                                name=f"v2_{t}_{pr}")
                    for pr in range(PAIRS)
                ]
                for t in range(T)
            ]
            for t in range(T):
                for pr in range(PAIRS):
                    nc.gpsimd.memset(v2[t][pr][:, :, C:C + 1], ZCOL)

            acc = consts.tile([P, NCH, C], BF16, tag="acc", name="acc")

            # ---- P16 = G16^T xs  -> fp8 [128, 2, N] ----
            p8 = consts.tile([P, 2, N], F8, tag="p8", name="p8")
            for bc in range(2):
                pp = ps_st.tile([P, N], F32, tag="st", name=f"pp{bc}")
                for nh in range(2):
                    nc.tensor.matmul(
                        pp[:, nh * 512:(nh + 1) * 512],
                        px8[:, :, bc * P:(bc + 1) * P],
                        px8[:, :, C + nh * 512:C + (nh + 1) * 512],
                        start=True, stop=True, perf_mode=DR,
                    )
                for nh in range(2):
                    eng = nc.vector.tensor_copy if bc == 0 else nc.scalar.copy
                    eng(
                        p8[:, bc, nh * 512:(nh + 1) * 512],
                        pp[:, nh * 512:(nh + 1) * 512],
                    )

            e2 = [[None] * PAIRS for _ in range(T)]

            def emit_v(t):
                for pr in range(PAIRS):
                    vp = ps_v.tile([P, 2 * C], F32, tag="vp", name=f"vp{t}{pr}")
                    for j in range(2):
                        mi = 2 * pr + j
                        nc.tensor.matmul(
                            vp[:, j * C:(j + 1) * C],
                            xt8[t][:, :, mi * P:(mi + 1) * P],
                            wv8,
                            start=True, stop=True, perf_mode=DR,
                        )
                    for j in range(2):
                        nc.vector.tensor_copy(
                            v2[t][pr][:, j, 0:C], vp[:, j * C:(j + 1) * C]
                        )

            def emit_s(t, mi_list):
                for mi in mi_list:
                    st = ps_st.tile([P, N], F32, tag="st", name=f"st{t}_{mi}")
                    for nh in range(2):
                        nc.tensor.matmul(
                            st[:, nh * 512:(nh + 1) * 512],
                            xt8[t][:, :, mi * P:(mi + 1) * P],
                            p8[:, :, nh * 512:(nh + 1) * 512],
                            start=True, stop=True, perf_mode=DR,
                        )
                    pr, j = mi // 2, mi % 2
                    if e2[t][pr] is None:
                        e2[t][pr] = e2pool.tile([P, 2, N], F8E5, tag="e2",
                                                name=f"e{t}_{pr}")
                    idx = t * MC + mi
                    if (t, mi) in OFFLOAD:
                        # exp via e5m2 bit-trick on DVE: byte =
                        # trunc(max(arg*4*log2e + 60.67, 0)) reinterpreted
                        y4 = ypool.tile([P, N], F32, tag="y4",
                                        name=f"y{t}_{mi}")
                        nc.vector.tensor_scalar(
                            y4, st, K1, kb_sb[:, 24 + idx:24 + idx + 1],
                            AOT.mult, AOT.add,
                        )
                        nc.vector.tensor_scalar_max(
                            e2[t][pr][:, j, :].bitcast(mybir.dt.uint8),
                            y4, 0.0,
                        )
                    else:
                        nc.scalar.activation(
                            e2[t][pr][:, j, :],
                            st,
                            func=mybir.ActivationFunctionType.Exp,
                            bias=kb_sb[:, idx:idx + 1],
                            scale=SCALE,
                        )

            def emit_o_group(t, ncx, act_assist=False, gps_add=False):
                otp = ps_o.tile([P, C + 1], F32, tag="otp",
                                name=f"o{t}_{ncx}")
                for pr in range(PAIRS):
                    nc.tensor.matmul(
                        otp,
                        e2[t][pr][:, :, ncx * P:(ncx + 1) * P],
                        v2[t][pr],
                        start=(pr == 0), stop=(pr == PAIRS - 1),
                        perf_mode=DR,
                    )
                r = rpool.tile([P, 1], F32, tag="r", name=f"r{t}_{ncx}")
                nc.vector.reciprocal(r, otp[:, C:C + 1])
                base = xsbv[:, ncx, :] if t == 0 else acc[:, ncx, :]
                if act_assist:
                    # post-exp tail: ACT is idle, split mul (ACT) + add
                    tmp = rpool.tile([P, C], BF16, tag="tmp", name=f"tm{ncx}")
                    nc.scalar.activation(
                        tmp, otp[:, 0:C],
                        func=mybir.ActivationFunctionType.Copy, scale=r,
                    )
                    add_eng = nc.gpsimd if gps_add else nc.vector
                    add_eng.tensor_add(acc[:, ncx, :], tmp, base)
                else:
                    nc.vector.scalar_tensor_tensor(
                        out=acc[:, ncx, :],
                        in0=otp[:, 0:C],
                        scalar=r,
                        in1=base,
                        op0=AOT.mult,
                        op1=AOT.add,
                    )

            # ---- software-pipelined teacher loop ----
            emit_v(0)
            emit_s(0, range(MC))
            for t in range(T):
                if t + 1 < T:
                    emit_v(t + 1)
                    emit_s(t + 1, [0, 1])
                    for k in range(NCH):
                        if k + 2 < MC:
                            emit_s(t + 1, [k + 2])
                        emit_o_group(t, k)
                else:
                    for k in range(NCH):
                        emit_o_group(t, k, act_assist=(k % 2 == 0),
                                     gps_add=(k in (0, 2)))
                        if k in (1, 3, 5):
                            nc.sync.dma_start(
                                out=out_d[:, k - 1:k + 1, :],
                                in_=acc[:, k - 1:k + 1, :],
                            )
                        elif k >= 6:
                            nc.sync.dma_start(
                                out=out_d[:, k:k + 1, :],
                                in_=acc[:, k:k + 1, :],
                            )

    _split_multi_waits(nc)
    if not nc.is_finalized():
        nc.finalize()
    return nc


def _split_multi_waits(nc):
    """walrus can encode at most one sync-wait per instruction. Hoist every
    wait of a multi-wait instruction onto single-wait nops on the same
    engine, placed immediately before it in program order."""
    fixes = []
    for fn in nc.m.functions:
        for blk in fn.blocks:
            for inst in blk.instructions:
                si = getattr(inst, "sync_info", None)
                if (si is not None and si.on_wait and len(si.on_wait) > 1
                        and getattr(inst, "engine", None) is not None):
                    fixes.append((blk, inst))
    for blk, inst in fixes:
        si = inst.sync_info
        waits = list(si.on_wait)
        nops = []
        for w in waits:
            nop = nc.engines[inst.engine].nop(nofuse=True).ins
            nop.sync_info = mybir.SyncInfo(on_wait=[w], on_update=[])
            nops.append(nop)
        inst.sync_info = mybir.SyncInfo(on_wait=[], on_update=list(si.on_update))
        nop_names = {n.name for n in nops}
        for fn2 in nc.m.functions:
            for blk2 in fn2.blocks:
                blk2.instructions = [
                    i for i in blk2.instructions if i.name not in nop_names
                ]
        pos = next(i for i, x in enumerate(blk.instructions)
                   if x.name == inst.name)
        blk.instructions = (blk.instructions[:pos] + nops
                            + blk.instructions[pos:])


_NC = None


def _get_nc():
    global _NC
    if _NC is None:
        _NC = build_nc()
    return _NC


FP8 = ml_dtypes.float8_e4m3
NPBF16 = ml_dtypes.bfloat16


def _pair2(a):
    """[256, X...] -> [128, 2, X...] with rows (k, k+128) paired."""
    return np.ascontiguousarray(
        a.reshape(2, P, *a.shape[1:]).swapaxes(0, 1)
    )


def make_in_maps(student_feat, t_feat0, t_feat1, t_feat2,
                 Wq, bq, Wk, bk, Wv, bv):
    del bk  # constant per softmax column -> cancels
    Wq = np.asarray(Wq, np.float64)
    Wk = np.asarray(Wk, np.float64)
    Wv = np.asarray(Wv, np.float64)
    bq = np.asarray(bq, np.float64)
    bv = np.asarray(bv, np.float32)
    g16 = (16.0 * (Wq.T @ Wk)).astype(np.float32)
    u16 = (16.0 * (Wk.T @ bq)).astype(np.float32)
    wvT16 = (16.0 * Wv.T).astype(np.float32)

    g8 = _pair2(g16).astype(FP8)
    wv8 = _pair2(wvT16).astype(FP8)

    xs = np.asarray(student_feat, np.float32).reshape(B, C, N)
    xt = np.stack(
        [np.asarray(t, np.float32) for t in (t_feat0, t_feat1, t_feat2)],
        axis=1,
    ).reshape(B, T, C, N)

    in_maps = []
    for b in range(B):
        px8 = np.concatenate([g8, _pair2(xs[b]).astype(FP8)], axis=2)
        xt8 = np.stack([_pair2(xt[b, t]) for t in range(T)]).astype(FP8)
        xsbv = np.ascontiguousarray(
            (xs[b].T + bv[None, :]).reshape(NCH, P, C).swapaxes(0, 1)
        ).astype(NPBF16)
        # per-key bias fold: kb_t[m] = (xt_t[:,m] . Wk^T bq)/16 - 2;
        # cols 24:48 hold the bit-trick variant 4*log2e*kb + 60.67
        kb = np.empty((P, 2 * T * MC), np.float32)
        for t in range(T):
            v = (xt[b, t].T @ (u16 / 256.0)) + EXP_SHIFT  # [N]
            kb[:, t * MC:(t + 1) * MC] = v.reshape(MC, P).T
        kb[:, 24:48] = kb[:, 0:24] * L2E4 + KB4_OFF
        in_maps.append({
            "px8": px8, "xt8": xt8, "wv8": wv8,
            "kb": kb, "xsbv": xsbv,
        })
    return in_maps


def run(in_maps, trace=False):
    nc = _get_nc()
    return run_bass_kernel_spmd(nc, in_maps, core_ids=list(range(B)),
                                trace=trace)


def gather_out(res):
    outs = []
    for b in range(B):
        o = np.asarray(res.results[b]["out"], np.float32)  # [128, 8, 256]
        o = o.swapaxes(0, 1).reshape(N, C)  # [n, c]
        outs.append(o.T.reshape(C, H, W))
    return np.stack(outs)


def kernel(student_feat, t_feat0, t_feat1, t_feat2,
           Wq, bq, Wk, bk, Wv, bv):
    in_maps = make_in_maps(student_feat, t_feat0, t_feat1, t_feat2,
                           Wq, bq, Wk, bk, Wv, bv)
    res = run(in_maps, trace=False)
    return gather_out(res).astype(np.float32)


# revision 14
# speedup vs baseline: 2.6141x; 1.0411x over previous
"""CrossTeacherAttention Trainium2 kernel (v2).

Math (per batch element b; xs/xt as [C=256, N=1024], N=H*W):
  S_t[n,m] = scale * (q_n . k_m) with q = Wq xs + bq, k_t = Wk xt_t + bk.
  bk-terms are constant per softmax column -> dropped (softmax invariant).
  S_t = xs^T (Wq^T Wk) xt_t + (Wk^T bq)^T xt_t.  Host folds the weights:
    G16 = 16 Wq^T Wk,  u16 = 16 Wk^T bq  (x16 keeps fp8 entries in normal
    range), so with P16 = G16^T xs the exp argument is
    (xt^T P16)[m,n]/256 + kb_t[m]  with kb_t = (xt_t^T u16)/256 - 2
  (-2 is a uniform shift so exp output fits fp8; cancels in softmax).
  attn.mean(-1) of a softmax is exactly 1/N -> teacher weights are 1/3.
  out = xs + bv + sum_t (E'_t V_t) / (3 Z'_t),  Z'_t[n] = sum_m E'_t[m,n].

Layout trick: O is computed with E' as the *stationary* matmul operand
(lhsT = E'[m,2,n], rhs = V^T[m,2,c]) so the output lands as [n_part, c_free],
where the softmax normalizer is a per-partition scalar.  V^T gets a 257th
column holding 48.0, so column 256 of the O accumulator is 48*Z'_t and
(1/48)*16(V-scale) = 1/3 folds the teacher weight into the reciprocal.
Z row-sums and the bias fold thus cost no extra PE streaming.

All matmuls run in fp8e4 DoubleRow (two 128-row k-tiles per pass).  exp runs
on ACT reading [128,1024] PSUM spans, writing fp8 E' directly.  Inputs are
pre-quantized/interleaved on the host ([128, 2, *] k-pair layout), which also
halves HBM traffic.  Residual path (xs^T + bv) ships as bf16; output is bf16
[n, c], transposed back on the host.

Sharding: data-parallel over batch, B=8 -> one batch element per core.
"""

import sys

sys.path.insert(0, "/opt/trn_rl_repo")

import ml_dtypes
import numpy as np

import concourse.bass as bass
import concourse.tile as tile
from concourse import mybir
from concourse.bass_utils import run_bass_kernel_spmd

B, C, H, W = 8, 256, 32, 32
N = H * W  # 1024
T = 3
P = 128
NCH = N // P  # 8 n-chunks
MC = N // P  # 8 m-chunks
PAIRS = MC // 2  # 4 m-chunk pairs (DoubleRow)
F32 = mybir.dt.float32
BF16 = mybir.dt.bfloat16
F8 = mybir.dt.float8e4
F8E5 = mybir.dt.float8e5
DR = mybir.MatmulPerfMode.DoubleRow
SCALE = 1.0 / 256.0  # 1/sqrt(C) / 16 (the x16 weight prescale)
EXP_SHIFT = -2.0
ZCOL = 48.0  # 16 (V prescale) * 3 (teacher weight)
L2E4 = 4.0 * 1.4426950408889634  # 4*log2(e): e5m2 exponent scale
K1 = L2E4 / 256.0
KB4_OFF = 60.0 + 0.67  # (15+bias)<<2 plus rounding/curvature compensation
# (t, mi) exp tiles computed on DVE via the e5m2 bit-trick instead of ACT
OFFLOAD = {(0, 6), (1, 3), (2, 2), (2, 5)}
AOT = mybir.AluOpType


def build_nc():
    nc = bass.Bass()
    px8_d = nc.dram_tensor("px8", [P, 2, C + N], F8, kind="ExternalInput")
    xt8_d = nc.dram_tensor("xt8", [T, P, 2, N], F8, kind="ExternalInput")
    wv8_d = nc.dram_tensor("wv8", [P, 2, C], F8, kind="ExternalInput")
    kb_d = nc.dram_tensor("kb", [P, 2 * T * MC], F32, kind="ExternalInput")
    xsbv_d = nc.dram_tensor("xsbv", [P, NCH, C], BF16, kind="ExternalInput")
    out_d = nc.dram_tensor("out", [P, NCH, C], BF16, kind="ExternalOutput")

    with tile.TileContext(nc) as tc:
        with (
            tc.tile_pool(name="consts", bufs=1) as consts,
            tc.tile_pool(name="e2pool", bufs=8) as e2pool,
            tc.tile_pool(name="rpool", bufs=4) as rpool,
            tc.tile_pool(name="ypool", bufs=2) as ypool,
            tc.tile_pool(name="ps_st", bufs=2, space="PSUM") as ps_st,
            tc.tile_pool(name="ps_v", bufs=1, space="PSUM") as ps_v,
            tc.tile_pool(name="ps_o", bufs=3, space="PSUM") as ps_o,
            nc.allow_low_precision(reason="fp8/bf16 kernel by design"),
        ):
            # ---- ACT table warmup: dummy exp so the exp/copy table
            # load happens during the input-DMA wait, off the critical path
            warm = consts.tile([P, 1], F32, tag="warm", name="warm")
            nc.gpsimd.memset(warm, 0.0)
            warm_o = consts.tile([P, 1], F32, tag="warm_o", name="warm_o")
            nc.scalar.activation(
                warm_o, warm, func=mybir.ActivationFunctionType.Exp, scale=0.0
            )

            # ---- input DMAs, in critical-path order ----
            px8 = consts.tile([P, 2, C + N], F8, tag="px8", name="px8")
            nc.sync.dma_start(out=px8[:, :, 0:C + 512],
                              in_=px8_d[:, :, 0:C + 512])
            nc.sync.dma_start(out=px8[:, :, C + 512:],
                              in_=px8_d[:, :, C + 512:])
            kb_sb = consts.tile([P, 2 * T * MC], F32, tag="kb", name="kb")
            nc.sync.dma_start(out=kb_sb, in_=kb_d[:, :])
            xt8 = []
            for t in range(T):
                xt = consts.tile([P, 2, N], F8, tag=f"xt{t}", name=f"xt{t}")
                xt8.append(xt)
            nc.sync.dma_start(out=xt8[0], in_=xt8_d[0])
            wv8 = consts.tile([P, 2, C], F8, tag="wv8", name="wv8")
            nc.sync.dma_start(out=wv8, in_=wv8_d[:, :, :])
            nc.sync.dma_start(out=xt8[1], in_=xt8_d[1])
            nc.sync.dma_start(out=xt8[2], in_=xt8_d[2])
            xsbv = consts.tile([P, NCH, C], BF16, tag="xsbv", name="xsbv")
            nc.sync.dma_start(out=xsbv, in_=xsbv_d[:, :, :])

            # V^T tiles (fp8) with the 48.0 normalizer column
            v2 = [
                [
                    consts.tile([P, 2, C + 1], F8, tag=f"v2_{t}_{pr}",
                                name=f"v2_{t}_{pr}")
                    for pr in range(PAIRS)
                ]
                for t in range(T)
            ]
            for t in range(T):
                for pr in range(PAIRS):
                    nc.gpsimd.memset(v2[t][pr][:, :, C:C + 1], ZCOL)

            acc = consts.tile([P, NCH, C], BF16, tag="acc", name="acc")

            # ---- P16 = G16^T xs  -> fp8 [128, 2, N] ----
            p8 = consts.tile([P, 2, N], F8, tag="p8", name="p8")
            for bc in range(2):
                pp = ps_st.tile([P, N], F32, tag="st", name=f"pp{bc}")
                for nh in range(2):
                    nc.tensor.matmul(
                        pp[:, nh * 512:(nh + 1) * 512],
                        px8[:, :, bc * P:(bc + 1) * P],
                        px8[:, :, C + nh * 512:C + (nh + 1) * 512],
                        start=True, stop=True, perf_mode=DR,
                    )
                for nh in range(2):
                    eng = nc.vector.tensor_copy if bc == 0 else nc.scalar.copy
                    eng(
                        p8[:, bc, nh * 512:(nh + 1) * 512],
                        pp[:, nh * 512:(nh + 1) * 512],
                    )

            e2 = [[None] * PAIRS for _ in range(T)]

            def emit_v(t):
                for pr in range(PAIRS):
                    vp = ps_v.tile([P, 2 * C], F32, tag="vp", name=f"vp{t}{pr}")
                    for j in range(2):
                        mi = 2 * pr + j
                        nc.tensor.matmul(
                            vp[:, j * C:(j + 1) * C],
                            xt8[t][:, :, mi * P:(mi + 1) * P],
                            wv8,
                            start=True, stop=True, perf_mode=DR,
                        )
                    nc.vector.tensor_copy(v2[t][pr][:, :, 0:C], vp)

            def emit_s(t, mi_list):
                for mi in mi_list:
                    st = ps_st.tile([P, N], F32, tag="st", name=f"st{t}_{mi}")
                    for nh in range(2):
                        nc.tensor.matmul(
                            st[:, nh * 512:(nh + 1) * 512],
                            xt8[t][:, :, mi * P:(mi + 1) * P],
                            p8[:, :, nh * 512:(nh + 1) * 512],
                            start=True, stop=True, perf_mode=DR,
                        )
                    pr, j = mi // 2, mi % 2
                    if e2[t][pr] is None:
                        e2[t][pr] = e2pool.tile([P, 2, N], F8E5, tag="e2",
                                                name=f"e{t}_{pr}")
                    idx = t * MC + mi
                    if (t, mi) in OFFLOAD:
                        # exp via e5m2 bit-trick on DVE: byte =
                        # trunc(max(arg*4*log2e + 60.67, 0)) reinterpreted
                        y4 = ypool.tile([P, N], F32, tag="y4",
                                        name=f"y{t}_{mi}")
                        nc.vector.tensor_scalar(
                            y4, st, K1, kb_sb[:, 24 + idx:24 + idx + 1],
                            AOT.mult, AOT.add,
                        )
                        nc.gpsimd.tensor_scalar_max(
                            e2[t][pr][:, j, :].bitcast(mybir.dt.uint8),
                            y4, 0.0,
                        )
                    else:
                        nc.scalar.activation(
                            e2[t][pr][:, j, :],
                            st,
                            func=mybir.ActivationFunctionType.Exp,
                            bias=kb_sb[:, idx:idx + 1],
                            scale=SCALE,
                        )

            def emit_o_group(t, ncx, act_assist=False, gps_add=False):
                otp = ps_o.tile([P, C + 1], F32, tag="otp",
                                name=f"o{t}_{ncx}")
                for pr in range(PAIRS):
                    nc.tensor.matmul(
                        otp,
                        e2[t][pr][:, :, ncx * P:(ncx + 1) * P],
                        v2[t][pr],
                        start=(pr == 0), stop=(pr == PAIRS - 1),
                        perf_mode=DR,
                    )
                r = rpool.tile([P, 1], F32, tag="r", name=f"r{t}_{ncx}")
                nc.vector.reciprocal(r, otp[:, C:C + 1])
                base = xsbv[:, ncx, :] if t == 0 else acc[:, ncx, :]
                if act_assist:
                    # post-exp tail: ACT is idle, split mul (ACT) + add
                    tmp = rpool.tile([P, C], BF16, tag="tmp", name=f"tm{ncx}")
                    nc.scalar.activation(
                        tmp, otp[:, 0:C],
                        func=mybir.ActivationFunctionType.Copy, scale=r,
                    )
                    add_eng = nc.gpsimd if gps_add else nc.vector
                    add_eng.tensor_add(acc[:, ncx, :], tmp, base)
                else:
                    nc.vector.scalar_tensor_tensor(
                        out=acc[:, ncx, :],
                        in0=otp[:, 0:C],
                        scalar=r,
                        in1=base,
                        op0=AOT.mult,
                        op1=AOT.add,
                    )

            # ---- software-pipelined teacher loop ----
            emit_v(0)
            emit_s(0, range(MC))
            for t in range(T):
                if t + 1 < T:
                    emit_v(t + 1)
                    emit_s(t + 1, [0, 1])
                    for k in range(NCH):
                        if k + 2 < MC:
                            emit_s(t + 1, [k + 2])
                        emit_o_group(t, k)
                else:
                    for k in range(NCH):
                        emit_o_group(t, k, act_assist=(k % 2 == 0),
                                     gps_add=(k in (0, 2)))
                        if k in (1, 3, 5):
                            nc.sync.dma_start(
                                out=out_d[:, k - 1:k + 1, :],
                                in_=acc[:, k - 1:k + 1, :],
                            )
                        elif k >= 6:
                            nc.sync.dma_start(
                                out=out_d[:, k:k + 1, :],
                                in_=acc[:, k:k + 1, :],
                            )

    _split_multi_waits(nc)
    if not nc.is_finalized():
        nc.finalize()
    return nc


def _split_multi_waits(nc):
    """walrus can encode at most one sync-wait per instruction. Hoist every
    wait of a multi-wait instruction onto single-wait nops on the same
    engine, placed immediately before it in program order."""
    fixes = []
    for fn in nc.m.functions:
        for blk in fn.blocks:
            for inst in blk.instructions:
                si = getattr(inst, "sync_info", None)
                if (si is not None and si.on_wait and len(si.on_wait) > 1
                        and getattr(inst, "engine", None) is not None):
                    fixes.append((blk, inst))
    for blk, inst in fixes:
        si = inst.sync_info
        waits = list(si.on_wait)
        nops = []
        for w in waits:
            nop = nc.engines[inst.engine].nop(nofuse=True).ins
            nop.sync_info = mybir.SyncInfo(on_wait=[w], on_update=[])
            nops.append(nop)
        inst.sync_info = mybir.SyncInfo(on_wait=[], on_update=list(si.on_update))
        nop_names = {n.name for n in nops}
        for fn2 in nc.m.functions:
            for blk2 in fn2.blocks:
                blk2.instructions = [
                    i for i in blk2.instructions if i.name not in nop_names
                ]
        pos = next(i for i, x in enumerate(blk.instructions)
                   if x.name == inst.name)
        blk.instructions = (blk.instructions[:pos] + nops
                            + blk.instructions[pos:])


_NC = None


def _get_nc():
    global _NC
    if _NC is None:
        _NC = build_nc()
    return _NC


FP8 = ml_dtypes.float8_e4m3
NPBF16 = ml_dtypes.bfloat16


def _pair2(a):
    """[256, X...] -> [128, 2, X...] with rows (k, k+128) paired."""
    return np.ascontiguousarray(
        a.reshape(2, P, *a.shape[1:]).swapaxes(0, 1)
    )


def make_in_maps(student_feat, t_feat0, t_feat1, t_feat2,
                 Wq, bq, Wk, bk, Wv, bv):
    del bk  # constant per softmax column -> cancels
    Wq = np.asarray(Wq, np.float64)
    Wk = np.asarray(Wk, np.float64)
    Wv = np.asarray(Wv, np.float64)
    bq = np.asarray(bq, np.float64)
    bv = np.asarray(bv, np.float32)
    g16 = (16.0 * (Wq.T @ Wk)).astype(np.float32)
    u16 = (16.0 * (Wk.T @ bq)).astype(np.float32)
    wvT16 = (16.0 * Wv.T).astype(np.float32)

    g8 = _pair2(g16).astype(FP8)
    wv8 = _pair2(wvT16).astype(FP8)

    xs = np.asarray(student_feat, np.float32).reshape(B, C, N)
    xt = np.stack(
        [np.asarray(t, np.float32) for t in (t_feat0, t_feat1, t_feat2)],
        axis=1,
    ).reshape(B, T, C, N)

    in_maps = []
    for b in range(B):
        px8 = np.concatenate([g8, _pair2(xs[b]).astype(FP8)], axis=2)
        xt8 = np.stack([_pair2(xt[b, t]) for t in range(T)]).astype(FP8)
        xsbv = np.ascontiguousarray(
            (xs[b].T + bv[None, :]).reshape(NCH, P, C).swapaxes(0, 1)
        ).astype(NPBF16)
        # per-key bias fold: kb_t[m] = (xt_t[:,m] . Wk^T bq)/16 - 2;
        # cols 24:48 hold the bit-trick variant 4*log2e*kb + 60.67
        kb = np.empty((P, 2 * T * MC), np.float32)
        for t in range(T):
            v = (xt[b, t].T @ (u16 / 256.0)) + EXP_SHIFT  # [N]
            kb[:, t * MC:(t + 1) * MC] = v.reshape(MC, P).T
        kb[:, 24:48] = kb[:, 0:24] * L2E4 + KB4_OFF
        in_maps.append({
            "px8": px8, "xt8": xt8, "wv8": wv8,
            "kb": kb, "xsbv": xsbv,
        })
    return in_maps


def run(in_maps, trace=False):
    nc = _get_nc()
    return run_bass_kernel_spmd(nc, in_maps, core_ids=list(range(B)),
                                trace=trace)


def gather_out(res):
    outs = []
    for b in range(B):
        o = np.asarray(res.results[b]["out"], np.float32)  # [128, 8, 256]
        o = o.swapaxes(0, 1).reshape(N, C)  # [n, c]
        outs.append(o.T.reshape(C, H, W))
    return np.stack(outs)


def kernel(student_feat, t_feat0, t_feat1, t_feat2,
           Wq, bq, Wk, bk, Wv, bv):
    in_maps = make_in_maps(student_feat, t_feat0, t_feat1, t_feat2,
                           Wq, bq, Wk, bk, Wv, bv)
    res = run(in_maps, trace=False)
    return gather_out(res).astype(np.float32)


# revision 18
# speedup vs baseline: 2.7287x; 1.0438x over previous
"""CrossTeacherAttention Trainium2 kernel (v2).

Math (per batch element b; xs/xt as [C=256, N=1024], N=H*W):
  S_t[n,m] = scale * (q_n . k_m) with q = Wq xs + bq, k_t = Wk xt_t + bk.
  bk-terms are constant per softmax column -> dropped (softmax invariant).
  S_t = xs^T (Wq^T Wk) xt_t + (Wk^T bq)^T xt_t.  Host folds the weights:
    G16 = 16 Wq^T Wk,  u16 = 16 Wk^T bq  (x16 keeps fp8 entries in normal
    range), so with P16 = G16^T xs the exp argument is
    (xt^T P16)[m,n]/256 + kb_t[m]  with kb_t = (xt_t^T u16)/256 - 2
  (-2 is a uniform shift so exp output fits fp8; cancels in softmax).
  attn.mean(-1) of a softmax is exactly 1/N -> teacher weights are 1/3.
  out = xs + bv + sum_t (E'_t V_t) / (3 Z'_t),  Z'_t[n] = sum_m E'_t[m,n].

Layout trick: O is computed with E' as the *stationary* matmul operand
(lhsT = E'[m,2,n], rhs = V^T[m,2,c]) so the output lands as [n_part, c_free],
where the softmax normalizer is a per-partition scalar.  V^T gets a 257th
column holding 48.0, so column 256 of the O accumulator is 48*Z'_t and
(1/48)*16(V-scale) = 1/3 folds the teacher weight into the reciprocal.
Z row-sums and the bias fold thus cost no extra PE streaming.

All matmuls run in fp8e4 DoubleRow (two 128-row k-tiles per pass).  exp runs
on ACT reading [128,1024] PSUM spans, writing fp8 E' directly.  Inputs are
pre-quantized/interleaved on the host ([128, 2, *] k-pair layout), which also
halves HBM traffic.  Residual path (xs^T + bv) ships as bf16; output is bf16
[n, c], transposed back on the host.

Sharding: data-parallel over batch, B=8 -> one batch element per core.
"""

import sys

sys.path.insert(0, "/opt/trn_rl_repo")

import ml_dtypes
import numpy as np

import concourse.bass as bass
import concourse.tile as tile
from concourse import mybir
from concourse.bass_utils import run_bass_kernel_spmd

B, C, H, W = 8, 256, 32, 32
N = H * W  # 1024
T = 3
P = 128
NCH = N // P  # 8 n-chunks
MC = N // P  # 8 m-chunks
PAIRS = MC // 2  # 4 m-chunk pairs (DoubleRow)
F32 = mybir.dt.float32
BF16 = mybir.dt.bfloat16
F8 = mybir.dt.float8e4
F8E5 = mybir.dt.float8e5
DR = mybir.MatmulPerfMode.DoubleRow
SCALE = 1.0 / 256.0  # 1/sqrt(C) / 16 (the x16 weight prescale)
EXP_SHIFT = -2.0
ZCOL = 48.0  # 16 (V prescale) * 3 (teacher weight)
L2E4 = 4.0 * 1.4426950408889634  # 4*log2(e): e5m2 exponent scale
K1 = L2E4 / 256.0
KB4_OFF = 60.0 + 0.67  # (15+bias)<<2 plus rounding/curvature compensation
# (t, mi) exp tiles computed on DVE via the e5m2 bit-trick instead of ACT
import ast as _ast
import os as _os
OFFLOAD = set(
    _ast.literal_eval(_os.environ.get("KOFFLOAD", "[(0,4),(0,6),(1,3),(1,6),(2,3),(2,5)]"))
)
AOT = mybir.AluOpType


def build_nc():
    nc = bass.Bass()
    px8_d = nc.dram_tensor("px8", [P, 2, C + N], F8, kind="ExternalInput")
    xt8_d = nc.dram_tensor("xt8", [T, P, 2, N], F8, kind="ExternalInput")
    wv8_d = nc.dram_tensor("wv8", [P, 2, C], F8, kind="ExternalInput")
    kb_d = nc.dram_tensor("kb", [P, 2 * T * MC], F32, kind="ExternalInput")
    xsbv_d = nc.dram_tensor("xsbv", [P, NCH, C], BF16, kind="ExternalInput")
    out_d = nc.dram_tensor("out", [P, NCH, C], BF16, kind="ExternalOutput")

    with tile.TileContext(nc) as tc:
        with (
            tc.tile_pool(name="consts", bufs=1) as consts,
            tc.tile_pool(name="e2pool", bufs=8) as e2pool,
            tc.tile_pool(name="rpool", bufs=4) as rpool,
            tc.tile_pool(name="ypool", bufs=2) as ypool,
            tc.tile_pool(name="ps_st", bufs=2, space="PSUM") as ps_st,
            tc.tile_pool(name="ps_v", bufs=1, space="PSUM") as ps_v,
            tc.tile_pool(name="ps_o", bufs=3, space="PSUM") as ps_o,
            nc.allow_low_precision(reason="fp8/bf16 kernel by design"),
        ):
            # ---- ACT table warmup: dummy exp so the exp/copy table
            # load happens during the input-DMA wait, off the critical path
            warm = consts.tile([P, 1], F32, tag="warm", name="warm")
            nc.gpsimd.memset(warm, 0.0)
            warm_o = consts.tile([P, 1], F32, tag="warm_o", name="warm_o")
            nc.scalar.activation(
                warm_o, warm, func=mybir.ActivationFunctionType.Exp, scale=0.0
            )

            # ---- input DMAs, in critical-path order ----
            px8 = consts.tile([P, 2, C + N], F8, tag="px8", name="px8")
            nc.sync.dma_start(out=px8[:, :, 0:C + 512],
                              in_=px8_d[:, :, 0:C + 512])
            nc.sync.dma_start(out=px8[:, :, C + 512:],
                              in_=px8_d[:, :, C + 512:])
            kb_sb = consts.tile([P, 2 * T * MC], F32, tag="kb", name="kb")
            nc.sync.dma_start(out=kb_sb, in_=kb_d[:, :])
            xt8 = []
            for t in range(T):
                xt = consts.tile([P, 2, N], F8, tag=f"xt{t}", name=f"xt{t}")
                xt8.append(xt)
            nc.sync.dma_start(out=xt8[0], in_=xt8_d[0])
            wv8 = consts.tile([P, 2, C], F8, tag="wv8", name="wv8")
            nc.sync.dma_start(out=wv8, in_=wv8_d[:, :, :])
            nc.sync.dma_start(out=xt8[1], in_=xt8_d[1])
            nc.sync.dma_start(out=xt8[2], in_=xt8_d[2])
            xsbv = consts.tile([P, NCH, C], BF16, tag="xsbv", name="xsbv")
            nc.sync.dma_start(out=xsbv, in_=xsbv_d[:, :, :])

            # V^T tiles (fp8) with the 48.0 normalizer column
            v2 = [
                [
                    consts.tile([P, 2, C + 1], F8, tag=f"v2_{t}_{pr}",
                                name=f"v2_{t}_{pr}")
                    for pr in range(PAIRS)
                ]
                for t in range(T)
            ]
            for t in range(T):
                for pr in range(PAIRS):
                    nc.gpsimd.memset(v2[t][pr][:, :, C:C + 1], ZCOL)

            acc = consts.tile([P, NCH, C], BF16, tag="acc", name="acc")

            # ---- P16 = G16^T xs  -> fp8 [128, 2, N] ----
            # nh-major so each half's convert (DVE for bc0, ACT for bc1)
            # starts as soon as that half of the xs DMA lands
            p8 = consts.tile([P, 2, N], F8, tag="p8", name="p8")
            pp = [ps_st.tile([P, N], F32, tag="st", name=f"pp{bc}")
                  for bc in range(2)]
            for nh in range(2):
                for bc in range(2):
                    nc.tensor.matmul(
                        pp[bc][:, nh * 512:(nh + 1) * 512],
                        px8[:, :, bc * P:(bc + 1) * P],
                        px8[:, :, C + nh * 512:C + (nh + 1) * 512],
                        start=True, stop=True, perf_mode=DR,
                    )
                    eng = nc.vector.tensor_copy if bc == 0 else nc.scalar.copy
                    eng(
                        p8[:, bc, nh * 512:(nh + 1) * 512],
                        pp[bc][:, nh * 512:(nh + 1) * 512],
                    )

            e2 = [[None] * PAIRS for _ in range(T)]

            def emit_v(t):
                for pr in range(PAIRS):
                    vp = ps_v.tile([P, 2 * C], F32, tag="vp", name=f"vp{t}{pr}")
                    for j in range(2):
                        mi = 2 * pr + j
                        nc.tensor.matmul(
                            vp[:, j * C:(j + 1) * C],
                            xt8[t][:, :, mi * P:(mi + 1) * P],
                            wv8,
                            start=True, stop=True, perf_mode=DR,
                        )
                    nc.vector.tensor_copy(v2[t][pr][:, :, 0:C], vp)

            def emit_s(t, mi_list):
                for mi in mi_list:
                    st = ps_st.tile([P, N], F32, tag="st", name=f"st{t}_{mi}")
                    for nh in range(2):
                        nc.tensor.matmul(
                            st[:, nh * 512:(nh + 1) * 512],
                            xt8[t][:, :, mi * P:(mi + 1) * P],
                            p8[:, :, nh * 512:(nh + 1) * 512],
                            start=True, stop=True, perf_mode=DR,
                        )
                    pr, j = mi // 2, mi % 2
                    if e2[t][pr] is None:
                        e2[t][pr] = e2pool.tile([P, 2, N], F8E5, tag="e2",
                                                name=f"e{t}_{pr}")
                    idx = t * MC + mi
                    if (t, mi) in OFFLOAD:
                        # exp via e5m2 bit-trick on DVE: byte =
                        # trunc(max(arg*4*log2e + 60.67, 0)) reinterpreted
                        y4 = ypool.tile([P, N], F32, tag="y4",
                                        name=f"y{t}_{mi}")
                        nc.vector.tensor_scalar(
                            y4, st, K1, kb_sb[:, 24 + idx:24 + idx + 1],
                            AOT.mult, AOT.add,
                        )
                        nc.gpsimd.tensor_scalar_max(
                            e2[t][pr][:, j, :].bitcast(mybir.dt.uint8),
                            y4, 0.0,
                        )
                    else:
                        nc.scalar.activation(
                            e2[t][pr][:, j, :],
                            st,
                            func=mybir.ActivationFunctionType.Exp,
                            bias=kb_sb[:, idx:idx + 1],
                            scale=SCALE,
                        )

            def emit_o_group(t, ncx, act_assist=False, gps_add=False):
                otp = ps_o.tile([P, C + 1], F32, tag="otp",
                                name=f"o{t}_{ncx}")
                for pr in range(PAIRS):
                    nc.tensor.matmul(
                        otp,
                        e2[t][pr][:, :, ncx * P:(ncx + 1) * P],
                        v2[t][pr],
                        start=(pr == 0), stop=(pr == PAIRS - 1),
                        perf_mode=DR,
                    )
                r = rpool.tile([P, 1], F32, tag="r", name=f"r{t}_{ncx}")
                nc.vector.reciprocal(r, otp[:, C:C + 1])
                base = xsbv[:, ncx, :] if t == 0 else acc[:, ncx, :]
                if act_assist:
                    # post-exp tail: ACT is idle, split mul (ACT) + add
                    tmp = rpool.tile([P, C], BF16, tag="tmp", name=f"tm{ncx}")
                    nc.scalar.activation(
                        tmp, otp[:, 0:C],
                        func=mybir.ActivationFunctionType.Copy, scale=r,
                    )
                    add_eng = nc.gpsimd if gps_add else nc.vector
                    add_eng.tensor_add(acc[:, ncx, :], tmp, base)
                else:
                    nc.vector.scalar_tensor_tensor(
                        out=acc[:, ncx, :],
                        in0=otp[:, 0:C],
                        scalar=r,
                        in1=base,
                        op0=AOT.mult,
                        op1=AOT.add,
                    )

            # ---- software-pipelined teacher loop ----
            emit_v(0)
            emit_s(0, range(MC))
            for t in range(T):
                if t + 1 < T:
                    emit_v(t + 1)
                    emit_s(t + 1, [0, 1])
                    for k in range(NCH):
                        if k + 2 < MC:
                            emit_s(t + 1, [k + 2])
                        # if exp(t+1, k+2) is DVE-offloaded, the ACT bubble
                        # it opens aligns here: fill it with the normalize
                        assist = (t + 1, k + 2) in OFFLOAD
                        emit_o_group(t, k, act_assist=assist, gps_add=assist)
                else:
                    for k in range(NCH):
                        emit_o_group(t, k, act_assist=(k % 2 == 0),
                                     gps_add=(k in (0, 2)))
                        if k in (1, 3, 5):
                            nc.sync.dma_start(
                                out=out_d[:, k - 1:k + 1, :],
                                in_=acc[:, k - 1:k + 1, :],
                            )
                        elif k >= 6:
                            nc.sync.dma_start(
                                out=out_d[:, k:k + 1, :],
                                in_=acc[:, k:k + 1, :],
                            )

    _split_multi_waits(nc)
    if not nc.is_finalized():
        nc.finalize()
    return nc


def _split_multi_waits(nc):
    """walrus can encode at most one sync-wait per instruction. Hoist every
    wait of a multi-wait instruction onto single-wait nops on the same
    engine, placed immediately before it in program order."""
    fixes = []
    for fn in nc.m.functions:
        for blk in fn.blocks:
            for inst in blk.instructions:
                si = getattr(inst, "sync_info", None)
                if (si is not None and si.on_wait and len(si.on_wait) > 1
                        and getattr(inst, "engine", None) is not None):
                    fixes.append((blk, inst))
    for blk, inst in fixes:
        si = inst.sync_info
        waits = list(si.on_wait)
        nops = []
        for w in waits:
            nop = nc.engines[inst.engine].nop(nofuse=True).ins
            nop.sync_info = mybir.SyncInfo(on_wait=[w], on_update=[])
            nops.append(nop)
        inst.sync_info = mybir.SyncInfo(on_wait=[], on_update=list(si.on_update))
        nop_names = {n.name for n in nops}
        for fn2 in nc.m.functions:
            for blk2 in fn2.blocks:
                blk2.instructions = [
                    i for i in blk2.instructions if i.name not in nop_names
                ]
        pos = next(i for i, x in enumerate(blk.instructions)
                   if x.name == inst.name)
        blk.instructions = (blk.instructions[:pos] + nops
                            + blk.instructions[pos:])


_NC = None


def _get_nc():
    global _NC
    if _NC is None:
        _NC = build_nc()
    return _NC


FP8 = ml_dtypes.float8_e4m3
NPBF16 = ml_dtypes.bfloat16


def _pair2(a):
    """[256, X...] -> [128, 2, X...] with rows (k, k+128) paired."""
    return np.ascontiguousarray(
        a.reshape(2, P, *a.shape[1:]).swapaxes(0, 1)
    )


def make_in_maps(student_feat, t_feat0, t_feat1, t_feat2,
                 Wq, bq, Wk, bk, Wv, bv):
    del bk  # constant per softmax column -> cancels
    Wq = np.asarray(Wq, np.float64)
    Wk = np.asarray(Wk, np.float64)
    Wv = np.asarray(Wv, np.float64)
    bq = np.asarray(bq, np.float64)
    bv = np.asarray(bv, np.float32)
    g16 = (16.0 * (Wq.T @ Wk)).astype(np.float32)
    u16 = (16.0 * (Wk.T @ bq)).astype(np.float32)
    wvT16 = (16.0 * Wv.T).astype(np.float32)

    g8 = _pair2(g16).astype(FP8)
    wv8 = _pair2(wvT16).astype(FP8)

    xs = np.asarray(student_feat, np.float32).reshape(B, C, N)
    xt = np.stack(
        [np.asarray(t, np.float32) for t in (t_feat0, t_feat1, t_feat2)],
        axis=1,
    ).reshape(B, T, C, N)

    in_maps = []
    for b in range(B):
        px8 = np.concatenate([g8, _pair2(xs[b]).astype(FP8)], axis=2)
        xt8 = np.stack([_pair2(xt[b, t]) for t in range(T)]).astype(FP8)
        xsbv = np.ascontiguousarray(
            (xs[b].T + bv[None, :]).reshape(NCH, P, C).swapaxes(0, 1)
        ).astype(NPBF16)
        # per-key bias fold: kb_t[m] = (xt_t[:,m] . Wk^T bq)/16 - 2;
        # cols 24:48 hold the bit-trick variant 4*log2e*kb + 60.67
        kb = np.empty((P, 2 * T * MC), np.float32)
        for t in range(T):
            v = (xt[b, t].T @ (u16 / 256.0)) + EXP_SHIFT  # [N]
            kb[:, t * MC:(t + 1) * MC] = v.reshape(MC, P).T
        kb[:, 24:48] = kb[:, 0:24] * L2E4 + KB4_OFF
        in_maps.append({
            "px8": px8, "xt8": xt8, "wv8": wv8,
            "kb": kb, "xsbv": xsbv,
        })
    return in_maps


def run(in_maps, trace=False):
    nc = _get_nc()
    return run_bass_kernel_spmd(nc, in_maps, core_ids=list(range(B)),
                                trace=trace)


def gather_out(res):
    outs = []
    for b in range(B):
        o = np.asarray(res.results[b]["out"], np.float32)  # [128, 8, 256]
        o = o.swapaxes(0, 1).reshape(N, C)  # [n, c]
        outs.append(o.T.reshape(C, H, W))
    return np.stack(outs)


def kernel(student_feat, t_feat0, t_feat1, t_feat2,
           Wq, bq, Wk, bk, Wv, bv):
    in_maps = make_in_maps(student_feat, t_feat0, t_feat1, t_feat2,
                           Wq, bq, Wk, bk, Wv, bv)
    res = run(in_maps, trace=False)
    return gather_out(res).astype(np.float32)
